# revision 6
# baseline (speedup 1.0000x reference)
"""GCN (2-layer GCNConv + question encoder) on 8 Trainium2 NeuronCores.

Strategy (self-contained, full inputs in / full outputs out):
  - Nodes partitioned contiguously across 8 cores (6272 = 49*128 per core,
    last core padded). Edges (with self-loops appended) are partitioned by
    destination node and sorted by destination on the host; each dst-tile of
    128 nodes gets its edge list padded to a uniform chunks-per-tile (CPT)
    so all cores run one SPMD program.
  - GCN identity used: D^-1/2 A D^-1/2 (h W) == scatter-add over edges of
    table[src] scaled by dinv[dst], where table = (dinv * h) @ W.
    The table is built shard-wise (dense matmul via a DMA-transposed fp16
    copy of dinv*h) and AllGathered to every core's HBM.
  - Per-edge gather uses the custom dma_gather instruction (int16 indices,
    base pointer placed mid-table so signed offsets cover all 50176 rows).
  - Scatter-add runs on the TensorEngine: per 128-edge chunk a one-hot
    [edge, dst_local] selection matrix (built by DVE iota-compare) is
    matmul-accumulated against the gathered messages in PSUM.
  - Epilogue per dst tile: out = psum * dinv[dst] + b (+ relu for layer 1),
    all f32. Messages/tables are fp16 (abs-max relative error ~2.6e-4).
  - The question encoder is data-parallel over rows, computed in f32 via
    PE-transpose + matmul, overlapped with the table AllGather.
"""
import sys

sys.path.insert(0, "/opt/trn_rl_repo")

import numpy as np
import ml_dtypes

import concourse.bacc as bacc
import concourse.mybir as mybir
import concourse.tile as tile
from concourse import bass
from concourse.bass_utils import run_bass_kernel_spmd
from concourse.library_config import mlp
from concourse.masks import make_identity

F32 = mybir.dt.float32
F16 = mybir.dt.float16
I16 = mybir.dt.int16
I32 = mybir.dt.int32

N_CORES = 8
N = 50000
D = 128
NT = 49                   # dst tiles per core
NPC = NT * 128            # nodes per core (padded)
TBL_ROWS = N_CORES * NPC  # 50176
BASE = 25000              # gather base row (signed int16 offsets)
GATHER_IDX_CAP = 12288    # Q7 scratch limit: <=~16k idx per dma_gather call
NQT = 20                  # q tiles per core
QPC = NQT * 128           # q rows per core (padded)
NQ = 20000

_cache = {}


def _build_nc(cpt: int):
    """Build and compile the SPMD kernel for a given chunks-per-tile."""
    epad = NT * cpt * 128  # padded edges per core

    nc = bacc.Bacc("TRN2", target_bir_lowering=False, debug=False,
                   num_devices=N_CORES)
    dt = mybir.dt

    # ---- I/O -----------------------------------------------------------
    x_sh = nc.dram_tensor("x_sh", [NPC, D], F32, kind="ExternalInput")
    q_sh = nc.dram_tensor("q_sh", [QPC, D], F32, kind="ExternalInput")
    w1 = nc.dram_tensor("w1", [D, D], F32, kind="ExternalInput")
    w2 = nc.dram_tensor("w2", [D, D], F32, kind="ExternalInput")
    wq = nc.dram_tensor("wq", [D, D], F32, kind="ExternalInput")
    b1 = nc.dram_tensor("b1", [1, D], F32, kind="ExternalInput")
    b2 = nc.dram_tensor("b2", [1, D], F32, kind="ExternalInput")
    bq = nc.dram_tensor("bq", [1, D], F32, kind="ExternalInput")
    iota_in = nc.dram_tensor("iota", [128, 128], F16, kind="ExternalInput")
    rpa = nc.dram_tensor("rpa", [128, NT], I32, kind="ExternalInput")
    rpb = nc.dram_tensor("rpb", [128, NT], I32, kind="ExternalInput")
    idx16 = nc.dram_tensor("idx16", [128, epad // 16], I16, kind="ExternalInput")
    dstloc = nc.dram_tensor("dstloc", [128, epad // 128], F16, kind="ExternalInput")

    h2_out = nc.dram_tensor("h2_out", [NPC, D], F32, kind="ExternalOutput")
    q_out = nc.dram_tensor("q_out", [QPC, D], F32, kind="ExternalOutput")
    import os
    _dbg = bool(os.environ.get("KERNEL_DEBUG"))
    if _dbg:
        dbg_tbl = nc.dram_tensor("dbg_tbl", [TBL_ROWS, D], F16, kind="ExternalOutput")
        dbg_g1 = nc.dram_tensor("dbg_g1", [NPC, D], F16, kind="ExternalOutput")
        dbg_dinv = nc.dram_tensor("dbg_dinv", [128, NT], F32, kind="ExternalOutput")

    with tile.TileContext(nc) as tc:
        with (
            tc.tile_pool(name="const", bufs=1) as cpool,
            tc.tile_pool(name="work", bufs=3) as wpool,
            tc.tile_pool(name="gath", bufs=2) as gpool,
            tc.tile_pool(name="oh", bufs=3) as ohpool,
            tc.tile_pool(name="psA", bufs=3, space="PSUM") as psA,
            tc.tile_pool(name="psB", bufs=4, space="PSUM") as psB,
            tc.tile_pool(name="dram", bufs=1, space="DRAM") as dram,
        ):
            nc.gpsimd.load_library(mlp)

            # ---- constants ------------------------------------------------
            iota_t = cpool.tile([128, 128], F16)
            nc.sync.dma_start(iota_t[:], iota_in[:, :])
            ident = cpool.tile([128, 128], F32)
            make_identity(nc, ident[:])
            ones_row = cpool.tile([1, 128], F32)
            nc.vector.memset(ones_row[:], 1.0)

            def load_w16(w):
                wf = wpool.tile([128, 128], F32, name="wf32")
                nc.sync.dma_start(wf[:], w[:, :])
                wh = cpool.tile([128, 128], F16, name=f"wh_{w.name}")
                nc.vector.tensor_copy(wh[:], wf[:])
                return wh

            w1h = load_w16(w1)
            w2h = load_w16(w2)
            wq_t = cpool.tile([128, 128], F32)
            nc.sync.dma_start(wq_t[:], wq[:, :])

            def bias_bcast(b):
                brow = wpool.tile([1, 128], F32, name="brow")
                nc.sync.dma_start(brow[:], b[:, :])
                ps = psB.tile([128, 128], F32, space="PSUM", name="mps")
                nc.tensor.matmul(ps[:], lhsT=ones_row[:], rhs=brow[:],
                                 start=True, stop=True)
                bb = cpool.tile([128, 128], F32, name=f"bb_{b.name}")
                nc.vector.tensor_copy(bb[:], ps[:])
                return bb

            b1_bc = bias_bcast(b1)
            b2_bc = bias_bcast(b2)
            bq_bc = bias_bcast(bq)

            # ---- dinv = rsqrt(deg), deg = rowptr diff + 1 (self loop) -----
            ra = wpool.tile([128, NT], I32, name="ra")
            rb = wpool.tile([128, NT], I32, name="rb")
            nc.sync.dma_start(ra[:], rpa[:, :])
            nc.sync.dma_start(rb[:], rpb[:, :])
            degi = wpool.tile([128, NT], I32, name="degi")
            nc.vector.tensor_tensor(out=degi[:], in0=rb[:], in1=ra[:],
                                    op=mybir.AluOpType.subtract)
            # rowptr spans the self-loop-augmented edge list, so the diff
            # already includes the self loop — just convert int32 -> f32.
            deg = wpool.tile([128, NT], F32, name="degf")
            nc.vector.tensor_copy(deg[:], degi[:])
            dinv = cpool.tile([128, NT], F32)
            rec = wpool.tile([128, NT], F32, name="rec")
            nc.vector.reciprocal(rec[:], deg[:])
            nc.scalar.activation(dinv[:], rec[:],
                                 mybir.ActivationFunctionType.Sqrt)
            # two Newton iterations: y <- y*(1.5 - 0.5*d*y*y)
            for _ in range(2):
                t1 = wpool.tile([128, NT], F32, name="nwt1")
                nc.vector.tensor_tensor(out=t1[:], in0=dinv[:], in1=dinv[:],
                                        op=mybir.AluOpType.mult)
                nc.vector.tensor_tensor(out=t1[:], in0=t1[:], in1=deg[:],
                                        op=mybir.AluOpType.mult)
                nc.vector.tensor_scalar(out=t1[:], in0=t1[:], scalar1=-0.5,
                                        scalar2=1.5,
                                        op0=mybir.AluOpType.mult,
                                        op1=mybir.AluOpType.add)
                nc.vector.tensor_tensor(out=dinv[:], in0=dinv[:], in1=t1[:],
                                        op=mybir.AluOpType.mult)

            # ---- DRAM intermediates --------------------------------------
            g_hbm = [dram.tile([NPC, D], F16, name=f"g{i}") for i in range(2)]
            tbl_shard = [dram.tile([NPC, D], F16, name=f"ts{i}") for i in range(2)]
            tbl_full = [dram.tile([TBL_ROWS, D], F16, name=f"tf{i}") for i in range(2)]

            def build_table(layer, src_rows):
                """table = (dinv * src_rows) @ W, sharded + allgathered.

                src_rows: callable t -> AP of [128,128] f32 rows in SBUF
                (already scaled by dinv for layer 1; for layer 0 loads x)."""
                gh = g_hbm[layer]
                for t in range(NT):
                    g16 = wpool.tile([128, 128], F16, name="g16")
                    src_rows(t, g16)
                    nc.sync.dma_start(gh[t * 128:(t + 1) * 128, :], g16[:])
                gT = cpool.tile([128, NPC], F16, name=f"gT{layer}")
                nc.sync.dma_start(gT[:], gh[:, :], transpose=True)
                wh = w1h if layer == 0 else w2h
                for t in range(NT):
                    ps = psB.tile([128, 128], F32, space="PSUM", name="mps")
                    nc.tensor.matmul(ps[:], lhsT=gT[:, t * 128:(t + 1) * 128],
                                     rhs=wh[:], start=True, stop=True)
                    th = wpool.tile([128, 128], F16, name="th")
                    nc.vector.tensor_copy(th[:], ps[:])
                    nc.sync.dma_start(tbl_shard[layer][t * 128:(t + 1) * 128, :],
                                      th[:])
                nc.gpsimd.collective_compute(
                    "AllGather", mybir.AluOpType.bypass,
                    replica_groups=[list(range(N_CORES))],
                    ins=[tbl_shard[layer].opt()],
                    outs=[tbl_full[layer].opt()],
                )

            # ---- table for layer 1: (dinv * x) @ W1 ----------------------
            def x_rows(t, out16):
                xt = wpool.tile([128, 128], F32, name="xt")
                nc.sync.dma_start(xt[:], x_sh[t * 128:(t + 1) * 128, :])
                nc.vector.tensor_scalar(out=out16[:], in0=xt[:],
                                        scalar1=dinv[:, t:t + 1], scalar2=None,
                                        op0=mybir.AluOpType.mult)

            build_table(0, x_rows)

            # ---- question encoder (overlaps the AllGather) ---------------
            for t in range(NQT):
                qt = wpool.tile([128, 128], F32, name="qt")
                nc.sync.dma_start(qt[:], q_sh[t * 128:(t + 1) * 128, :])
                qT_ps = psB.tile([128, 128], F32, space="PSUM", name="mps")
                nc.tensor.transpose(out=qT_ps[:], in_=qt[:], identity=ident[:])
                qT = wpool.tile([128, 128], F32, name="qT")
                nc.vector.tensor_copy(qT[:], qT_ps[:])
                qo_ps = psB.tile([128, 128], F32, space="PSUM", name="mps")
                nc.tensor.matmul(qo_ps[:], lhsT=qT[:], rhs=wq_t[:],
                                 start=True, stop=True)
                qo = wpool.tile([128, 128], F32, name="qo")
                nc.vector.tensor_tensor(out=qo[:], in0=qo_ps[:], in1=bq_bc[:],
                                        op=mybir.AluOpType.add)
                nc.sync.dma_start(q_out[t * 128:(t + 1) * 128, :], qo[:])

            # ---- load edge metadata (once) -------------------------------
            idx_sb = cpool.tile([128, epad // 16], I16)
            nc.sync.dma_start(idx_sb[:], idx16[:, :])
            dst_sb = cpool.tile([128, epad // 128], F16)
            nc.sync.dma_start(dst_sb[:], dstloc[:, :])

            # ---- GCN layers ----------------------------------------------
            def gcn_layer(layer):
                tbl = tbl_full[layer]
                h1_rows = []  # layer 0: stash h1 tiles for table-2 build
                gt = max(1, GATHER_IDX_CAP // (cpt * 128))
                for g0 in range(0, NT, gt):
                    ntile = min(gt, NT - g0)
                    ncall = ntile * cpt * 128
                    gath = gpool.tile([128, gt * cpt, 128], F16, name="gath")
                    nc.gpsimd.dma_gather(
                        gath[:, :ntile * cpt, :],
                        tbl[BASE:, :],
                        idx_sb[:, (g0 * cpt * 128) // 16:
                                  ((g0 + ntile) * cpt * 128) // 16],
                        ncall, ncall, 128,
                        single_packet=False,
                    )
                    for ti in range(ntile):
                        t = g0 + ti
                        oh = ohpool.tile([128, cpt * 128], F16, name="oh")
                        nc.vector.tensor_tensor(
                            out=oh[:].rearrange("p (c j) -> p c j", j=128),
                            in0=dst_sb[:, t * cpt:(t + 1) * cpt]
                                .unsqueeze(-1).to_broadcast([128, cpt, 128]),
                            in1=iota_t[:].unsqueeze(1)
                                .to_broadcast([128, cpt, 128]),
                            op=mybir.AluOpType.is_equal,
                        )
                        ps = psA.tile([128, 128], F32, space="PSUM", name="agg")
                        for c in range(cpt):
                            nc.tensor.matmul(
                                ps[:],
                                lhsT=oh[:, c * 128:(c + 1) * 128],
                                rhs=gath[:, ti * cpt + c, :],
                                start=(c == 0), stop=(c == cpt - 1),
                            )
                        # epilogue
                        o = wpool.tile([128, 128], F32, name="epi_o")
                        nc.vector.tensor_scalar(
                            out=o[:], in0=ps[:], scalar1=dinv[:, t:t + 1],
                            scalar2=None, op0=mybir.AluOpType.mult)
                        bb = b1_bc if layer == 0 else b2_bc
                        nc.vector.tensor_tensor(out=o[:], in0=o[:], in1=bb[:],
                                                op=mybir.AluOpType.add)
                        if layer == 0:
                            h = wpool.tile([128, 128], F32, name="h1")
                            nc.scalar.activation(
                                h[:], o[:], mybir.ActivationFunctionType.Relu)
                            g16 = wpool.tile([128, 128], F16, name="g16b")
                            nc.vector.tensor_scalar(
                                out=g16[:], in0=h[:],
                                scalar1=dinv[:, t:t + 1], scalar2=None,
                                op0=mybir.AluOpType.mult)
                            nc.sync.dma_start(
                                g_hbm[1][t * 128:(t + 1) * 128, :], g16[:])
                        else:
                            nc.sync.dma_start(
                                h2_out[t * 128:(t + 1) * 128, :], o[:])
                return h1_rows

            gcn_layer(0)

            if _dbg:
                dbt = wpool.tile([128, NT], F32, name="dbt")
                nc.vector.tensor_copy(dbt[:], dinv[:])
                nc.sync.dma_start(dbg_dinv[:, :], dbt[:])
                for t in range(0, TBL_ROWS // 128):
                    tt = wpool.tile([128, 128], F16, name="dtt")
                    nc.sync.dma_start(tt[:], tbl_full[0][t * 128:(t + 1) * 128, :])
                    nc.sync.dma_start(dbg_tbl[t * 128:(t + 1) * 128, :], tt[:])
                for t in range(NT):
                    tg = wpool.tile([128, 128], F16, name="dtg")
                    nc.sync.dma_start(tg[:], g_hbm[1][t * 128:(t + 1) * 128, :])
                    nc.sync.dma_start(dbg_g1[t * 128:(t + 1) * 128, :], tg[:])

            # table for layer 2 from g_hbm[1] (already dinv*relu(h1), f16)
            gT2 = cpool.tile([128, NPC], F16)
            nc.sync.dma_start(gT2[:], g_hbm[1][:, :], transpose=True)
            for t in range(NT):
                ps = psB.tile([128, 128], F32, space="PSUM", name="mps")
                nc.tensor.matmul(ps[:], lhsT=gT2[:, t * 128:(t + 1) * 128],
                                 rhs=w2h[:], start=True, stop=True)
                th = wpool.tile([128, 128], F16, name="th2")
                nc.vector.tensor_copy(th[:], ps[:])
                nc.sync.dma_start(tbl_shard[1][t * 128:(t + 1) * 128, :], th[:])
            nc.gpsimd.collective_compute(
                "AllGather", mybir.AluOpType.bypass,
                replica_groups=[list(range(N_CORES))],
                ins=[tbl_shard[1].opt()],
                outs=[tbl_full[1].opt()],
            )

            gcn_layer(1)

    nc.compile()
    return nc


def _wrap16(a, cols):
    """[n] int16 -> [128, n/16]: element i at partition i%16 (replicated x8)."""
    w = a.reshape(cols, 16).T
    return np.tile(w, (8, 1)).copy()


def _prep(x, edge_index, q_emb, W1, b1, W2, b2, Wq, bq):
    src = np.asarray(edge_index[0], dtype=np.int64)
    dst = np.asarray(edge_index[1], dtype=np.int64)
    # self loops as real edges
    loop = np.arange(N, dtype=np.int64)
    s_all = np.concatenate([src, loop])
    d_all = np.concatenate([dst, loop])
    order = np.argsort(d_all, kind="stable")
    s_srt = s_all[order]
    d_srt = d_all[order]
    rowptr = np.searchsorted(d_srt, np.arange(N + 1))

    # per-global-tile edge counts -> uniform chunks per tile (always >=1 pad)
    gt_starts = rowptr[np.minimum(np.arange(0, TBL_ROWS + 1, 128), N)]
    counts = gt_starts[1:] - gt_starts[:-1]  # length 392 incl dummy tiles
    cpt = int(np.max(counts) // 128 + 1)
    epad = NT * cpt * 128

    ins = []
    x = np.asarray(x, dtype=np.float32)
    q = np.asarray(q_emb, dtype=np.float32)
    iota = np.broadcast_to(np.arange(128, dtype=np.float32), (128, 128))
    iota16 = iota.astype(np.float16)

    for k in range(N_CORES):
        idx = np.zeros((NT, cpt * 128), dtype=np.int16)
        dl = np.full((NT, cpt * 128), -1.0, dtype=np.float16)
        for t in range(NT):
            g = k * NT + t
            a = min(g * 128, N)
            b = min(g * 128 + 128, N)
            e0, e1 = rowptr[a], rowptr[b]
            cnt = e1 - e0
            idx[t, :cnt] = (s_srt[e0:e1] - BASE).astype(np.int16)
            dl[t, :cnt] = (d_srt[e0:e1] - g * 128).astype(np.float16)

        # rowptr pairs for degree (dummy nodes -> deg 1)
        node_ids = np.minimum(k * NPC + np.arange(NPC), N)
        ra = rowptr[node_ids].astype(np.int32)
        rb = rowptr[np.minimum(node_ids + 1, N)].astype(np.int32)

        xs = np.zeros((NPC, D), np.float32)
        nlo, nhi = k * NPC, min((k + 1) * NPC, N)
        xs[: nhi - nlo] = x[nlo:nhi]
        qs = np.zeros((QPC, D), np.float32)
        qlo, qhi = k * QPC, min((k + 1) * QPC, NQ)
        qs[: qhi - qlo] = q[qlo:qhi]

        ins.append({
            "x_sh": xs,
            "q_sh": qs,
            "w1": np.asarray(W1, np.float32),
            "w2": np.asarray(W2, np.float32),
            "wq": np.asarray(Wq, np.float32),
            "b1": np.asarray(b1, np.float32).reshape(1, D),
            "b2": np.asarray(b2, np.float32).reshape(1, D),
            "bq": np.asarray(bq, np.float32).reshape(1, D),
            "iota": iota16,
            "rpa": ra.reshape(NT, 128).T.copy(),
            "rpb": rb.reshape(NT, 128).T.copy(),
            "idx16": _wrap16(idx.reshape(-1), epad // 16),
            "dstloc": dl.reshape(-1).reshape(epad // 128, 128).T.copy(),
        })
    return cpt, ins


def _run(inputs, trace=False):
    cpt, ins = _prep(**inputs)
    import os
    key = (cpt, bool(os.environ.get("KERNEL_DEBUG")))
    if key not in _cache:
        _cache[key] = _build_nc(cpt)
    nc = _cache[key]
    res = run_bass_kernel_spmd(nc, ins, core_ids=list(range(N_CORES)),
                               trace=trace)
    h2 = np.concatenate(
        [res.results[k]["h2_out"][: min((k + 1) * NPC, N) - k * NPC]
         for k in range(N_CORES)])
    ques = np.concatenate(
        [res.results[k]["q_out"][: min((k + 1) * QPC, NQ) - k * QPC]
         for k in range(N_CORES)])
    return (ques, h2), res


def kernel(**inputs):
    out, _ = _run(inputs, trace=False)
    return out


def kernel_traced(**inputs):
    return _run(inputs, trace=True)


# revision 7
# speedup vs baseline: 1.9247x; 1.9247x over previous
"""GCN (2-layer GCNConv + question encoder) on 8 Trainium2 NeuronCores.

Strategy (self-contained, full inputs in / full outputs out):
  - Nodes partitioned contiguously across 8 cores (6272 = 49*128 per core,
    last core padded). Edges (with self-loops appended) are partitioned by
    destination node and sorted by destination on the host; each dst-tile of
    128 nodes gets its edge list padded to a uniform chunks-per-tile (CPT)
    so all cores run one SPMD program.
  - GCN identity used: D^-1/2 A D^-1/2 (h W) == scatter-add over edges of
    table[src] scaled by dinv[dst], where table = (dinv * h) @ W.
    The table is built shard-wise (dense matmul via a DMA-transposed fp16
    copy of dinv*h) and AllGathered to every core's HBM.
  - Per-edge gather uses the custom dma_gather instruction (int16 indices,
    base pointer placed mid-table so signed offsets cover all 50176 rows).
  - Scatter-add runs on the TensorEngine: per 128-edge chunk a one-hot
    [edge, dst_local] selection matrix (built by DVE iota-compare) is
    matmul-accumulated against the gathered messages in PSUM.
  - Epilogue per dst tile: out = psum * dinv[dst] + b (+ relu for layer 1),
    all f32. Messages/tables are fp16 (abs-max relative error ~2.6e-4).
  - The question encoder is data-parallel over rows, computed in f32 via
    PE-transpose + matmul, overlapped with the table AllGather.
"""
import sys

sys.path.insert(0, "/opt/trn_rl_repo")

import numpy as np
import ml_dtypes

import concourse.bacc as bacc
import concourse.mybir as mybir
import concourse.tile as tile
from concourse import bass
from concourse.bass_utils import run_bass_kernel_spmd
from concourse.library_config import mlp
from concourse.masks import make_identity

F32 = mybir.dt.float32
F16 = mybir.dt.float16
I16 = mybir.dt.int16
I32 = mybir.dt.int32

N_CORES = 8
N = 50000
D = 128
NT = 49                   # dst tiles per core
NPC = NT * 128            # nodes per core (padded)
TBL_ROWS = N_CORES * NPC  # 50176
BASE = 25000              # gather base row (signed int16 offsets)
GATHER_IDX_CAP = 5120     # smaller calls spread across 4 SWDGE queues
NQT = 20                  # q tiles per core
QPC = NQT * 128           # q rows per core (padded)
NQ = 20000

_cache = {}


def _build_nc(cpt: int):
    """Build and compile the SPMD kernel for a given chunks-per-tile."""
    epad = NT * cpt * 128  # padded edges per core

    nc = bacc.Bacc("TRN2", target_bir_lowering=False, debug=False,
                   num_devices=N_CORES, num_swdge_queues=4)
    dt = mybir.dt

    # ---- I/O -----------------------------------------------------------
    x_sh = nc.dram_tensor("x_sh", [NPC, D], F32, kind="ExternalInput")
    q_sh = nc.dram_tensor("q_sh", [QPC, D], F32, kind="ExternalInput")
    w1 = nc.dram_tensor("w1", [D, D], F32, kind="ExternalInput")
    w2 = nc.dram_tensor("w2", [D, D], F32, kind="ExternalInput")
    wq = nc.dram_tensor("wq", [D, D], F32, kind="ExternalInput")
    b1 = nc.dram_tensor("b1", [1, D], F32, kind="ExternalInput")
    b2 = nc.dram_tensor("b2", [1, D], F32, kind="ExternalInput")
    bq = nc.dram_tensor("bq", [1, D], F32, kind="ExternalInput")
    iota_in = nc.dram_tensor("iota", [128, 128], F16, kind="ExternalInput")
    rpa = nc.dram_tensor("rpa", [128, NT], I32, kind="ExternalInput")
    rpb = nc.dram_tensor("rpb", [128, NT], I32, kind="ExternalInput")
    idx16 = nc.dram_tensor("idx16", [128, epad // 16], I16, kind="ExternalInput")
    dstloc = nc.dram_tensor("dstloc", [128, epad // 128], F16, kind="ExternalInput")

    h2_out = nc.dram_tensor("h2_out", [NPC, D], F32, kind="ExternalOutput")
    q_out = nc.dram_tensor("q_out", [QPC, D], F32, kind="ExternalOutput")
    import os
    _dbg = bool(os.environ.get("KERNEL_DEBUG"))
    if _dbg:
        dbg_tbl = nc.dram_tensor("dbg_tbl", [TBL_ROWS, D], F16, kind="ExternalOutput")
        dbg_g1 = nc.dram_tensor("dbg_g1", [NPC, D], F16, kind="ExternalOutput")
        dbg_dinv = nc.dram_tensor("dbg_dinv", [128, NT], F32, kind="ExternalOutput")

    with tile.TileContext(nc) as tc:
        with (
            tc.tile_pool(name="const", bufs=1) as cpool,
            tc.tile_pool(name="work", bufs=3) as wpool,
            tc.tile_pool(name="gath", bufs=6) as gpool,
            tc.tile_pool(name="oh", bufs=3) as ohpool,
            tc.tile_pool(name="psA", bufs=3, space="PSUM") as psA,
            tc.tile_pool(name="psB", bufs=4, space="PSUM") as psB,
            tc.tile_pool(name="dram", bufs=1, space="DRAM") as dram,
        ):
            nc.gpsimd.load_library(mlp)

            # ---- constants ------------------------------------------------
            iota_t = cpool.tile([128, 128], F16)
            nc.sync.dma_start(iota_t[:], iota_in[:, :])
            ident = cpool.tile([128, 128], F32)
            make_identity(nc, ident[:])
            ones_row = cpool.tile([1, 128], F32)
            nc.vector.memset(ones_row[:], 1.0)

            def load_w16(w):
                wf = wpool.tile([128, 128], F32, name="wf32")
                nc.sync.dma_start(wf[:], w[:, :])
                wh = cpool.tile([128, 128], F16, name=f"wh_{w.name}")
                nc.vector.tensor_copy(wh[:], wf[:])
                return wh

            w1h = load_w16(w1)
            w2h = load_w16(w2)
            wq_t = cpool.tile([128, 128], F32)
            nc.sync.dma_start(wq_t[:], wq[:, :])

            def bias_bcast(b):
                brow = wpool.tile([1, 128], F32, name="brow")
                nc.sync.dma_start(brow[:], b[:, :])
                ps = psB.tile([128, 128], F32, space="PSUM", name="mps")
                nc.tensor.matmul(ps[:], lhsT=ones_row[:], rhs=brow[:],
                                 start=True, stop=True)
                bb = cpool.tile([128, 128], F32, name=f"bb_{b.name}")
                nc.vector.tensor_copy(bb[:], ps[:])
                return bb

            b1_bc = bias_bcast(b1)
            b2_bc = bias_bcast(b2)
            bq_bc = bias_bcast(bq)

            # ---- dinv = rsqrt(deg), deg = rowptr diff + 1 (self loop) -----
            ra = wpool.tile([128, NT], I32, name="ra")
            rb = wpool.tile([128, NT], I32, name="rb")
            nc.sync.dma_start(ra[:], rpa[:, :])
            nc.sync.dma_start(rb[:], rpb[:, :])
            degi = wpool.tile([128, NT], I32, name="degi")
            nc.vector.tensor_tensor(out=degi[:], in0=rb[:], in1=ra[:],
                                    op=mybir.AluOpType.subtract)
            # rowptr spans the self-loop-augmented edge list, so the diff
            # already includes the self loop — just convert int32 -> f32.
            deg = wpool.tile([128, NT], F32, name="degf")
            nc.vector.tensor_copy(deg[:], degi[:])
            dinv = cpool.tile([128, NT], F32)
            rec = wpool.tile([128, NT], F32, name="rec")
            nc.vector.reciprocal(rec[:], deg[:])
            nc.scalar.activation(dinv[:], rec[:],
                                 mybir.ActivationFunctionType.Sqrt)
            # two Newton iterations: y <- y*(1.5 - 0.5*d*y*y)
            for _ in range(2):
                t1 = wpool.tile([128, NT], F32, name="nwt1")
                nc.vector.tensor_tensor(out=t1[:], in0=dinv[:], in1=dinv[:],
                                        op=mybir.AluOpType.mult)
                nc.vector.tensor_tensor(out=t1[:], in0=t1[:], in1=deg[:],
                                        op=mybir.AluOpType.mult)
                nc.vector.tensor_scalar(out=t1[:], in0=t1[:], scalar1=-0.5,
                                        scalar2=1.5,
                                        op0=mybir.AluOpType.mult,
                                        op1=mybir.AluOpType.add)
                nc.vector.tensor_tensor(out=dinv[:], in0=dinv[:], in1=t1[:],
                                        op=mybir.AluOpType.mult)

            # ---- DRAM intermediates --------------------------------------
            g_hbm = [dram.tile([NPC, D], F16, name=f"g{i}") for i in range(2)]
            tbl_shard = [dram.tile([NPC, D], F16, name=f"ts{i}") for i in range(2)]
            tbl_full = [dram.tile([TBL_ROWS, D], F16, name=f"tf{i}") for i in range(2)]

            def build_table(layer, src_rows):
                """table = (dinv * src_rows) @ W, sharded + allgathered.

                src_rows: callable t -> AP of [128,128] f32 rows in SBUF
                (already scaled by dinv for layer 1; for layer 0 loads x)."""
                gh = g_hbm[layer]
                for t in range(NT):
                    g16 = wpool.tile([128, 128], F16, name="g16")
                    src_rows(t, g16)
                    nc.sync.dma_start(gh[t * 128:(t + 1) * 128, :], g16[:])
                gT = cpool.tile([128, NPC], F16, name=f"gT{layer}")
                nc.sync.dma_start(gT[:], gh[:, :], transpose=True)
                wh = w1h if layer == 0 else w2h
                for t in range(NT):
                    ps = psB.tile([128, 128], F32, space="PSUM", name="mps")
                    nc.tensor.matmul(ps[:], lhsT=gT[:, t * 128:(t + 1) * 128],
                                     rhs=wh[:], start=True, stop=True)
                    th = wpool.tile([128, 128], F16, name="th")
                    nc.vector.tensor_copy(th[:], ps[:])
                    nc.sync.dma_start(tbl_shard[layer][t * 128:(t + 1) * 128, :],
                                      th[:])
                nc.gpsimd.collective_compute(
                    "AllGather", mybir.AluOpType.bypass,
                    replica_groups=[list(range(N_CORES))],
                    ins=[tbl_shard[layer].opt()],
                    outs=[tbl_full[layer].opt()],
                )

            # ---- table for layer 1: (dinv * x) @ W1 ----------------------
            def x_rows(t, out16):
                xt = wpool.tile([128, 128], F32, name="xt")
                nc.sync.dma_start(xt[:], x_sh[t * 128:(t + 1) * 128, :])
                nc.vector.tensor_scalar(out=out16[:], in0=xt[:],
                                        scalar1=dinv[:, t:t + 1], scalar2=None,
                                        op0=mybir.AluOpType.mult)

            build_table(0, x_rows)

            # ---- question encoder (overlaps the AllGather) ---------------
            for t in range(NQT):
                qt = wpool.tile([128, 128], F32, name="qt")
                nc.sync.dma_start(qt[:], q_sh[t * 128:(t + 1) * 128, :])
                qT_ps = psB.tile([128, 128], F32, space="PSUM", name="mps")
                nc.tensor.transpose(out=qT_ps[:], in_=qt[:], identity=ident[:])
                qT = wpool.tile([128, 128], F32, name="qT")
                nc.vector.tensor_copy(qT[:], qT_ps[:])
                qo_ps = psB.tile([128, 128], F32, space="PSUM", name="mps")
                nc.tensor.matmul(qo_ps[:], lhsT=qT[:], rhs=wq_t[:],
                                 start=True, stop=True)
                qo = wpool.tile([128, 128], F32, name="qo")
                nc.vector.tensor_tensor(out=qo[:], in0=qo_ps[:], in1=bq_bc[:],
                                        op=mybir.AluOpType.add)
                nc.sync.dma_start(q_out[t * 128:(t + 1) * 128, :], qo[:])

            # ---- load edge metadata (once) -------------------------------
            idx_sb = cpool.tile([128, epad // 16], I16)
            nc.sync.dma_start(idx_sb[:], idx16[:, :])
            dst_sb = cpool.tile([128, epad // 128], F16)
            nc.sync.dma_start(dst_sb[:], dstloc[:, :])

            # ---- GCN layers ----------------------------------------------
            def gcn_layer(layer):
                tbl = tbl_full[layer]
                h1_rows = []  # layer 0: stash h1 tiles for table-2 build
                gt = max(1, GATHER_IDX_CAP // (cpt * 128))
                for ci, g0 in enumerate(range(0, NT, gt)):
                    ntile = min(gt, NT - g0)
                    ncall = ntile * cpt * 128
                    gath = gpool.tile([128, gt * cpt, 128], F16, name="gath")
                    nc.gpsimd.dma_gather(
                        gath[:, :ntile * cpt, :],
                        tbl[BASE:, :],
                        idx_sb[:, (g0 * cpt * 128) // 16:
                                  ((g0 + ntile) * cpt * 128) // 16],
                        ncall, ncall, 128,
                        single_packet=False,
                        queue_num=ci % 4,
                    )
                    for ti in range(ntile):
                        t = g0 + ti
                        oh = ohpool.tile([128, cpt * 128], F16, name="oh")
                        nc.vector.tensor_tensor(
                            out=oh[:].rearrange("p (c j) -> p c j", j=128),
                            in0=dst_sb[:, t * cpt:(t + 1) * cpt]
                                .unsqueeze(-1).to_broadcast([128, cpt, 128]),
                            in1=iota_t[:].unsqueeze(1)
                                .to_broadcast([128, cpt, 128]),
                            op=mybir.AluOpType.is_equal,
                        )
                        ps = psA.tile([128, 128], F32, space="PSUM", name="agg")
                        for c in range(cpt):
                            nc.tensor.matmul(
                                ps[:],
                                lhsT=oh[:, c * 128:(c + 1) * 128],
                                rhs=gath[:, ti * cpt + c, :],
                                start=(c == 0), stop=(c == cpt - 1),
                            )
                        # epilogue
                        o = wpool.tile([128, 128], F32, name="epi_o")
                        nc.vector.tensor_scalar(
                            out=o[:], in0=ps[:], scalar1=dinv[:, t:t + 1],
                            scalar2=None, op0=mybir.AluOpType.mult)
                        bb = b1_bc if layer == 0 else b2_bc
                        nc.vector.tensor_tensor(out=o[:], in0=o[:], in1=bb[:],
                                                op=mybir.AluOpType.add)
                        if layer == 0:
                            h = wpool.tile([128, 128], F32, name="h1")
                            nc.scalar.activation(
                                h[:], o[:], mybir.ActivationFunctionType.Relu)
                            g16 = wpool.tile([128, 128], F16, name="g16b")
                            nc.vector.tensor_scalar(
                                out=g16[:], in0=h[:],
                                scalar1=dinv[:, t:t + 1], scalar2=None,
                                op0=mybir.AluOpType.mult)
                            nc.sync.dma_start(
                                g_hbm[1][t * 128:(t + 1) * 128, :], g16[:])
                        else:
                            nc.sync.dma_start(
                                h2_out[t * 128:(t + 1) * 128, :], o[:])
                return h1_rows

            gcn_layer(0)

            if _dbg:
                dbt = wpool.tile([128, NT], F32, name="dbt")
                nc.vector.tensor_copy(dbt[:], dinv[:])
                nc.sync.dma_start(dbg_dinv[:, :], dbt[:])
                for t in range(0, TBL_ROWS // 128):
                    tt = wpool.tile([128, 128], F16, name="dtt")
                    nc.sync.dma_start(tt[:], tbl_full[0][t * 128:(t + 1) * 128, :])
                    nc.sync.dma_start(dbg_tbl[t * 128:(t + 1) * 128, :], tt[:])
                for t in range(NT):
                    tg = wpool.tile([128, 128], F16, name="dtg")
                    nc.sync.dma_start(tg[:], g_hbm[1][t * 128:(t + 1) * 128, :])
                    nc.sync.dma_start(dbg_g1[t * 128:(t + 1) * 128, :], tg[:])

            # table for layer 2 from g_hbm[1] (already dinv*relu(h1), f16)
            gT2 = cpool.tile([128, NPC], F16)
            nc.sync.dma_start(gT2[:], g_hbm[1][:, :], transpose=True)
            for t in range(NT):
                ps = psB.tile([128, 128], F32, space="PSUM", name="mps")
                nc.tensor.matmul(ps[:], lhsT=gT2[:, t * 128:(t + 1) * 128],
                                 rhs=w2h[:], start=True, stop=True)
                th = wpool.tile([128, 128], F16, name="th2")
                nc.vector.tensor_copy(th[:], ps[:])
                nc.sync.dma_start(tbl_shard[1][t * 128:(t + 1) * 128, :], th[:])
            nc.gpsimd.collective_compute(
                "AllGather", mybir.AluOpType.bypass,
                replica_groups=[list(range(N_CORES))],
                ins=[tbl_shard[1].opt()],
                outs=[tbl_full[1].opt()],
            )

            gcn_layer(1)

    nc.compile()
    return nc


def _wrap16(a, cols):
    """[n] int16 -> [128, n/16]: element i at partition i%16 (replicated x8)."""
    w = a.reshape(cols, 16).T
    return np.tile(w, (8, 1)).copy()


def _prep(x, edge_index, q_emb, W1, b1, W2, b2, Wq, bq):
    src = np.asarray(edge_index[0], dtype=np.int64)
    dst = np.asarray(edge_index[1], dtype=np.int64)
    # self loops as real edges
    loop = np.arange(N, dtype=np.int64)
    s_all = np.concatenate([src, loop])
    d_all = np.concatenate([dst, loop])
    order = np.argsort(d_all, kind="stable")
    s_srt = s_all[order]
    d_srt = d_all[order]
    rowptr = np.searchsorted(d_srt, np.arange(N + 1))

    # per-global-tile edge counts -> uniform chunks per tile (always >=1 pad)
    gt_starts = rowptr[np.minimum(np.arange(0, TBL_ROWS + 1, 128), N)]
    counts = gt_starts[1:] - gt_starts[:-1]  # length 392 incl dummy tiles
    cpt = int(np.max(counts) // 128 + 1)
    epad = NT * cpt * 128

    ins = []
    x = np.asarray(x, dtype=np.float32)
    q = np.asarray(q_emb, dtype=np.float32)
    iota = np.broadcast_to(np.arange(128, dtype=np.float32), (128, 128))
    iota16 = iota.astype(np.float16)

    for k in range(N_CORES):
        idx = np.zeros((NT, cpt * 128), dtype=np.int16)
        dl = np.full((NT, cpt * 128), -1.0, dtype=np.float16)
        for t in range(NT):
            g = k * NT + t
            a = min(g * 128, N)
            b = min(g * 128 + 128, N)
            e0, e1 = rowptr[a], rowptr[b]
            cnt = e1 - e0
            idx[t, :cnt] = (s_srt[e0:e1] - BASE).astype(np.int16)
            dl[t, :cnt] = (d_srt[e0:e1] - g * 128).astype(np.float16)

        # rowptr pairs for degree (dummy nodes -> deg 1)
        node_ids = np.minimum(k * NPC + np.arange(NPC), N)
        ra = rowptr[node_ids].astype(np.int32)
        rb = rowptr[np.minimum(node_ids + 1, N)].astype(np.int32)

        xs = np.zeros((NPC, D), np.float32)
        nlo, nhi = k * NPC, min((k + 1) * NPC, N)
        xs[: nhi - nlo] = x[nlo:nhi]
        qs = np.zeros((QPC, D), np.float32)
        qlo, qhi = k * QPC, min((k + 1) * QPC, NQ)
        qs[: qhi - qlo] = q[qlo:qhi]

        ins.append({
            "x_sh": xs,
            "q_sh": qs,
            "w1": np.asarray(W1, np.float32),
            "w2": np.asarray(W2, np.float32),
            "wq": np.asarray(Wq, np.float32),
            "b1": np.asarray(b1, np.float32).reshape(1, D),
            "b2": np.asarray(b2, np.float32).reshape(1, D),
            "bq": np.asarray(bq, np.float32).reshape(1, D),
            "iota": iota16,
            "rpa": ra.reshape(NT, 128).T.copy(),
            "rpb": rb.reshape(NT, 128).T.copy(),
            "idx16": _wrap16(idx.reshape(-1), epad // 16),
            "dstloc": dl.reshape(-1).reshape(epad // 128, 128).T.copy(),
        })
    return cpt, ins


def _run(inputs, trace=False):
    cpt, ins = _prep(**inputs)
    import os
    key = (cpt, bool(os.environ.get("KERNEL_DEBUG")))
    if key not in _cache:
        _cache[key] = _build_nc(cpt)
    nc = _cache[key]
    res = run_bass_kernel_spmd(nc, ins, core_ids=list(range(N_CORES)),
                               trace=trace)
    h2 = np.concatenate(
        [res.results[k]["h2_out"][: min((k + 1) * NPC, N) - k * NPC]
         for k in range(N_CORES)])
    ques = np.concatenate(
        [res.results[k]["q_out"][: min((k + 1) * QPC, NQ) - k * QPC]
         for k in range(N_CORES)])
    return (ques, h2), res


def kernel(**inputs):
    out, _ = _run(inputs, trace=False)
    return out


def kernel_traced(**inputs):
    return _run(inputs, trace=True)


# revision 9
# speedup vs baseline: 2.0459x; 1.0629x over previous
"""GCN (2-layer GCNConv + question encoder) on 8 Trainium2 NeuronCores.

Strategy (self-contained, full inputs in / full outputs out):
  - Nodes partitioned contiguously across 8 cores (6272 = 49*128 per core,
    last core padded). Edges (with self-loops appended) are partitioned by
    destination node and sorted by destination on the host; each dst-tile of
    128 nodes gets its edge list padded to a uniform chunks-per-tile (CPT)
    so all cores run one SPMD program.
  - GCN identity used: D^-1/2 A D^-1/2 (h W) == scatter-add over edges of
    table[src] scaled by dinv[dst], where table = (dinv * h) @ W.
    The table is built shard-wise (dense matmul via a DMA-transposed fp16
    copy of dinv*h) and AllGathered to every core's HBM.
  - Per-edge gather uses the custom dma_gather instruction (int16 indices,
    base pointer placed mid-table so signed offsets cover all 50176 rows).
  - Scatter-add runs on the TensorEngine: per 128-edge chunk a one-hot
    [edge, dst_local] selection matrix (built by DVE iota-compare) is
    matmul-accumulated against the gathered messages in PSUM.
  - Epilogue per dst tile: out = psum * dinv[dst] + b (+ relu for layer 1),
    all f32. Messages/tables are fp16 (abs-max relative error ~2.6e-4).
  - The question encoder is data-parallel over rows, computed in f32 via
    PE-transpose + matmul, overlapped with the table AllGather.
"""
import sys

sys.path.insert(0, "/opt/trn_rl_repo")

import numpy as np
import ml_dtypes

import concourse.bacc as bacc
import concourse.mybir as mybir
import concourse.tile as tile
from concourse import bass
from concourse.bass_utils import run_bass_kernel_spmd
from concourse.library_config import mlp
from concourse.masks import make_identity

F32 = mybir.dt.float32
F16 = mybir.dt.float16
I16 = mybir.dt.int16
I32 = mybir.dt.int32

N_CORES = 8
N = 50000
D = 128
NT = 49                   # dst tiles per core
NPC = NT * 128            # nodes per core (padded)
TBL_ROWS = N_CORES * NPC  # 50176
BASE = 25000              # gather base row (signed int16 offsets)
GATHER_IDX_CAP = 5120     # smaller calls spread across 4 SWDGE queues
NQT = 20                  # q tiles per core
QPC = NQT * 128           # q rows per core (padded)
NQ = 20000

_cache = {}


def _build_nc(cpt: int):
    """Build and compile the SPMD kernel for a given chunks-per-tile."""
    epad = NT * cpt * 128  # padded edges per core

    nc = bacc.Bacc("TRN2", target_bir_lowering=False, debug=False,
                   num_devices=N_CORES, num_swdge_queues=4)
    dt = mybir.dt

    # ---- I/O -----------------------------------------------------------
    x_sh = nc.dram_tensor("x_sh", [NPC, D], F32, kind="ExternalInput")
    q_sh = nc.dram_tensor("q_sh", [QPC, D], F32, kind="ExternalInput")
    w1 = nc.dram_tensor("w1", [D, D], F32, kind="ExternalInput")
    w2 = nc.dram_tensor("w2", [D, D], F32, kind="ExternalInput")
    wq = nc.dram_tensor("wq", [D, D], F32, kind="ExternalInput")
    b1 = nc.dram_tensor("b1", [1, D], F32, kind="ExternalInput")
    b2 = nc.dram_tensor("b2", [1, D], F32, kind="ExternalInput")
    bq = nc.dram_tensor("bq", [1, D], F32, kind="ExternalInput")
    iota_in = nc.dram_tensor("iota", [128, 128], F16, kind="ExternalInput")
    rpa = nc.dram_tensor("rpa", [128, NT], I32, kind="ExternalInput")
    rpb = nc.dram_tensor("rpb", [128, NT], I32, kind="ExternalInput")
    idx16 = nc.dram_tensor("idx16", [128, epad // 16], I16, kind="ExternalInput")
    dstloc = nc.dram_tensor("dstloc", [128, epad // 128], F16, kind="ExternalInput")

    h2_out = nc.dram_tensor("h2_out", [NPC, D], F32, kind="ExternalOutput")
    q_out = nc.dram_tensor("q_out", [QPC, D], F32, kind="ExternalOutput")
    import os
    _dbg = bool(os.environ.get("KERNEL_DEBUG"))
    if _dbg:
        dbg_tbl = nc.dram_tensor("dbg_tbl", [TBL_ROWS, D], F16, kind="ExternalOutput")
        dbg_g1 = nc.dram_tensor("dbg_g1", [NPC, D], F16, kind="ExternalOutput")
        dbg_dinv = nc.dram_tensor("dbg_dinv", [128, NT], F32, kind="ExternalOutput")

    with tile.TileContext(nc) as tc:
        with (
            tc.tile_pool(name="const", bufs=1) as cpool,
            tc.tile_pool(name="work", bufs=3) as wpool,
            tc.tile_pool(name="gath", bufs=10) as gpool,
            tc.tile_pool(name="oh", bufs=3) as ohpool,
            tc.tile_pool(name="psA", bufs=3, space="PSUM") as psA,
            tc.tile_pool(name="psB", bufs=4, space="PSUM") as psB,
            tc.tile_pool(name="dram", bufs=1, space="DRAM") as dram,
        ):
            nc.gpsimd.load_library(mlp)

            # ---- constants ------------------------------------------------
            iota_t = cpool.tile([128, 128], F16)
            nc.sync.dma_start(iota_t[:], iota_in[:, :])
            ident = cpool.tile([128, 128], F32)
            make_identity(nc, ident[:])
            ones_row = cpool.tile([1, 128], F32)
            nc.vector.memset(ones_row[:], 1.0)

            def load_w16(w):
                wf = wpool.tile([128, 128], F32, name="wf32")
                nc.sync.dma_start(wf[:], w[:, :])
                wh = cpool.tile([128, 128], F16, name=f"wh_{w.name}")
                nc.vector.tensor_copy(wh[:], wf[:])
                return wh

            w1h = load_w16(w1)
            w2h = load_w16(w2)
            wq_t = cpool.tile([128, 128], F32)
            nc.sync.dma_start(wq_t[:], wq[:, :])

            def bias_bcast(b):
                brow = wpool.tile([1, 128], F32, name="brow")
                nc.sync.dma_start(brow[:], b[:, :])
                ps = psB.tile([128, 128], F32, space="PSUM", name="mps")
                nc.tensor.matmul(ps[:], lhsT=ones_row[:], rhs=brow[:],
                                 start=True, stop=True)
                bb = cpool.tile([128, 128], F32, name=f"bb_{b.name}")
                nc.vector.tensor_copy(bb[:], ps[:])
                return bb

            b1_bc = bias_bcast(b1)
            b2_bc = bias_bcast(b2)
            bq_bc = bias_bcast(bq)

            # ---- dinv = rsqrt(deg), deg = rowptr diff + 1 (self loop) -----
            ra = wpool.tile([128, NT], I32, name="ra")
            rb = wpool.tile([128, NT], I32, name="rb")
            nc.sync.dma_start(ra[:], rpa[:, :])
            nc.sync.dma_start(rb[:], rpb[:, :])
            degi = wpool.tile([128, NT], I32, name="degi")
            nc.vector.tensor_tensor(out=degi[:], in0=rb[:], in1=ra[:],
                                    op=mybir.AluOpType.subtract)
            # rowptr spans the self-loop-augmented edge list, so the diff
            # already includes the self loop — just convert int32 -> f32.
            deg = wpool.tile([128, NT], F32, name="degf")
            nc.vector.tensor_copy(deg[:], degi[:])
            dinv = cpool.tile([128, NT], F32)
            rec = wpool.tile([128, NT], F32, name="rec")
            nc.vector.reciprocal(rec[:], deg[:])
            nc.scalar.activation(dinv[:], rec[:],
                                 mybir.ActivationFunctionType.Sqrt)
            # two Newton iterations: y <- y*(1.5 - 0.5*d*y*y)
            for _ in range(2):
                t1 = wpool.tile([128, NT], F32, name="nwt1")
                nc.vector.tensor_tensor(out=t1[:], in0=dinv[:], in1=dinv[:],
                                        op=mybir.AluOpType.mult)
                nc.vector.tensor_tensor(out=t1[:], in0=t1[:], in1=deg[:],
                                        op=mybir.AluOpType.mult)
                nc.vector.tensor_scalar(out=t1[:], in0=t1[:], scalar1=-0.5,
                                        scalar2=1.5,
                                        op0=mybir.AluOpType.mult,
                                        op1=mybir.AluOpType.add)
                nc.vector.tensor_tensor(out=dinv[:], in0=dinv[:], in1=t1[:],
                                        op=mybir.AluOpType.mult)

            # ---- DRAM intermediates --------------------------------------
            g_hbm = [dram.tile([NPC, D], F16, name=f"g{i}") for i in range(2)]
            tbl_shard = [dram.tile([NPC, D], F16, name=f"ts{i}") for i in range(2)]
            tbl_full = [nc.dram_tensor(f"tf{i}", [TBL_ROWS, D], F16,
                                       addr_space="Shared").ap()
                        for i in range(2)]

            def build_table(layer, src_rows):
                """table = (dinv * src_rows) @ W, sharded + allgathered.

                src_rows: callable t -> AP of [128,128] f32 rows in SBUF
                (already scaled by dinv for layer 1; for layer 0 loads x)."""
                gh = g_hbm[layer]
                for t in range(NT):
                    g16 = wpool.tile([128, 128], F16, name="g16")
                    src_rows(t, g16)
                    nc.sync.dma_start(gh[t * 128:(t + 1) * 128, :], g16[:])
                gT = cpool.tile([128, NPC], F16, name=f"gT{layer}")
                nc.sync.dma_start(gT[:], gh[:, :], transpose=True)
                wh = w1h if layer == 0 else w2h
                for t in range(NT):
                    ps = psB.tile([128, 128], F32, space="PSUM", name="mps")
                    nc.tensor.matmul(ps[:], lhsT=gT[:, t * 128:(t + 1) * 128],
                                     rhs=wh[:], start=True, stop=True)
                    th = wpool.tile([128, 128], F16, name="th")
                    nc.vector.tensor_copy(th[:], ps[:])
                    nc.sync.dma_start(tbl_shard[layer][t * 128:(t + 1) * 128, :],
                                      th[:])
                nc.gpsimd.collective_compute(
                    "AllGather", mybir.AluOpType.bypass,
                    replica_groups=[list(range(N_CORES))],
                    ins=[tbl_shard[layer].opt()],
                    outs=[tbl_full[layer].opt()],
                )

            # ---- table for layer 1: (dinv * x) @ W1 ----------------------
            x_all = cpool.tile([128, NT * 128], F32)
            nc.sync.dma_start(
                x_all[:].rearrange("p (t d) -> p t d", d=128),
                x_sh.ap().rearrange("(t p) d -> p t d", p=128))

            def x_rows(t, out16):
                nc.vector.tensor_scalar(out=out16[:],
                                        in0=x_all[:, t * 128:(t + 1) * 128],
                                        scalar1=dinv[:, t:t + 1], scalar2=None,
                                        op0=mybir.AluOpType.mult)

            build_table(0, x_rows)

            # ---- question encoder (overlaps the AllGather) ---------------
            q_all = cpool.tile([128, NQT * 128], F32)
            nc.sync.dma_start(
                q_all[:].rearrange("p (t d) -> p t d", d=128),
                q_sh.ap().rearrange("(t p) d -> p t d", p=128))
            for t in range(NQT):
                qT_ps = psB.tile([128, 128], F32, space="PSUM", name="mps")
                nc.tensor.transpose(out=qT_ps[:],
                                    in_=q_all[:, t * 128:(t + 1) * 128],
                                    identity=ident[:])
                qT = wpool.tile([128, 128], F32, name="qT")
                nc.vector.tensor_copy(qT[:], qT_ps[:])
                qo_ps = psB.tile([128, 128], F32, space="PSUM", name="mps")
                nc.tensor.matmul(qo_ps[:], lhsT=qT[:], rhs=wq_t[:],
                                 start=True, stop=True)
                qo = wpool.tile([128, 128], F32, name="qo")
                nc.vector.tensor_tensor(out=qo[:], in0=qo_ps[:], in1=bq_bc[:],
                                        op=mybir.AluOpType.add)
                nc.sync.dma_start(q_out[t * 128:(t + 1) * 128, :], qo[:])

            # ---- load edge metadata (once) -------------------------------
            idx_sb = cpool.tile([128, epad // 16], I16)
            nc.sync.dma_start(idx_sb[:], idx16[:, :])
            dst_sb = cpool.tile([128, epad // 128], F16)
            nc.sync.dma_start(dst_sb[:], dstloc[:, :])

            # ---- GCN layers ----------------------------------------------
            def gcn_layer(layer):
                tbl = tbl_full[layer].tensor.ap()
                h1_rows = []  # layer 0: stash h1 tiles for table-2 build
                gt = max(1, GATHER_IDX_CAP // (cpt * 128))
                for ci, g0 in enumerate(range(0, NT, gt)):
                    ntile = min(gt, NT - g0)
                    ncall = ntile * cpt * 128
                    gath = gpool.tile([128, gt * cpt, 128], F16, name="gath")
                    nc.gpsimd.dma_gather(
                        gath[:, :ntile * cpt, :],
                        tbl[BASE:, :],
                        idx_sb[:, (g0 * cpt * 128) // 16:
                                  ((g0 + ntile) * cpt * 128) // 16],
                        ncall, ncall, 128,
                        single_packet=False,
                        queue_num=ci % 4,
                    )
                    for ti in range(ntile):
                        t = g0 + ti
                        oh = ohpool.tile([128, cpt * 128], F16, name="oh")
                        nc.vector.tensor_tensor(
                            out=oh[:].rearrange("p (c j) -> p c j", j=128),
                            in0=dst_sb[:, t * cpt:(t + 1) * cpt]
                                .unsqueeze(-1).to_broadcast([128, cpt, 128]),
                            in1=iota_t[:].unsqueeze(1)
                                .to_broadcast([128, cpt, 128]),
                            op=mybir.AluOpType.is_equal,
                        )
                        ps = psA.tile([128, 128], F32, space="PSUM", name="agg")
                        for c in range(cpt):
                            nc.tensor.matmul(
                                ps[:],
                                lhsT=oh[:, c * 128:(c + 1) * 128],
                                rhs=gath[:, ti * cpt + c, :],
                                start=(c == 0), stop=(c == cpt - 1),
                            )
                        # epilogue
                        o = wpool.tile([128, 128], F32, name="epi_o")
                        nc.vector.tensor_scalar(
                            out=o[:], in0=ps[:], scalar1=dinv[:, t:t + 1],
                            scalar2=None, op0=mybir.AluOpType.mult)
                        bb = b1_bc if layer == 0 else b2_bc
                        nc.vector.tensor_tensor(out=o[:], in0=o[:], in1=bb[:],
                                                op=mybir.AluOpType.add)
                        if layer == 0:
                            h = wpool.tile([128, 128], F32, name="h1")
                            nc.scalar.activation(
                                h[:], o[:], mybir.ActivationFunctionType.Relu)
                            g16 = wpool.tile([128, 128], F16, name="g16b")
                            nc.vector.tensor_scalar(
                                out=g16[:], in0=h[:],
                                scalar1=dinv[:, t:t + 1], scalar2=None,
                                op0=mybir.AluOpType.mult)
                            nc.sync.dma_start(
                                g_hbm[1][t * 128:(t + 1) * 128, :], g16[:])
                        else:
                            nc.sync.dma_start(
                                h2_out[t * 128:(t + 1) * 128, :], o[:])
                return h1_rows

            gcn_layer(0)

            if _dbg:
                dbt = wpool.tile([128, NT], F32, name="dbt")
                nc.vector.tensor_copy(dbt[:], dinv[:])
                nc.sync.dma_start(dbg_dinv[:, :], dbt[:])
                for t in range(0, TBL_ROWS // 128):
                    tt = wpool.tile([128, 128], F16, name="dtt")
                    nc.sync.dma_start(tt[:], tbl_full[0].tensor.ap()[t * 128:(t + 1) * 128, :])
                    nc.sync.dma_start(dbg_tbl[t * 128:(t + 1) * 128, :], tt[:])
                for t in range(NT):
                    tg = wpool.tile([128, 128], F16, name="dtg")
                    nc.sync.dma_start(tg[:], g_hbm[1][t * 128:(t + 1) * 128, :])
                    nc.sync.dma_start(dbg_g1[t * 128:(t + 1) * 128, :], tg[:])

            # table for layer 2 from g_hbm[1] (already dinv*relu(h1), f16)
            gT2 = cpool.tile([128, NPC], F16)
            nc.sync.dma_start(gT2[:], g_hbm[1][:, :], transpose=True)
            for t in range(NT):
                ps = psB.tile([128, 128], F32, space="PSUM", name="mps")
                nc.tensor.matmul(ps[:], lhsT=gT2[:, t * 128:(t + 1) * 128],
                                 rhs=w2h[:], start=True, stop=True)
                th = wpool.tile([128, 128], F16, name="th2")
                nc.vector.tensor_copy(th[:], ps[:])
                nc.sync.dma_start(tbl_shard[1][t * 128:(t + 1) * 128, :], th[:])
            nc.gpsimd.collective_compute(
                "AllGather", mybir.AluOpType.bypass,
                replica_groups=[list(range(N_CORES))],
                ins=[tbl_shard[1].opt()],
                outs=[tbl_full[1].opt()],
            )

            gcn_layer(1)

    nc.compile()
    return nc


def _wrap16(a, cols):
    """[n] int16 -> [128, n/16]: element i at partition i%16 (replicated x8)."""
    w = a.reshape(cols, 16).T
    return np.tile(w, (8, 1)).copy()


def _prep(x, edge_index, q_emb, W1, b1, W2, b2, Wq, bq):
    src = np.asarray(edge_index[0], dtype=np.int64)
    dst = np.asarray(edge_index[1], dtype=np.int64)
    # self loops as real edges
    loop = np.arange(N, dtype=np.int64)
    s_all = np.concatenate([src, loop])
    d_all = np.concatenate([dst, loop])
    order = np.argsort(d_all, kind="stable")
    s_srt = s_all[order]
    d_srt = d_all[order]
    rowptr = np.searchsorted(d_srt, np.arange(N + 1))

    # per-global-tile edge counts -> uniform chunks per tile (always >=1 pad)
    gt_starts = rowptr[np.minimum(np.arange(0, TBL_ROWS + 1, 128), N)]
    counts = gt_starts[1:] - gt_starts[:-1]  # length 392 incl dummy tiles
    cpt = int(np.max(counts) // 128 + 1)
    epad = NT * cpt * 128

    ins = []
    x = np.asarray(x, dtype=np.float32)
    q = np.asarray(q_emb, dtype=np.float32)
    iota = np.broadcast_to(np.arange(128, dtype=np.float32), (128, 128))
    iota16 = iota.astype(np.float16)

    for k in range(N_CORES):
        idx = np.zeros((NT, cpt * 128), dtype=np.int16)
        dl = np.full((NT, cpt * 128), -1.0, dtype=np.float16)
        for t in range(NT):
            g = k * NT + t
            a = min(g * 128, N)
            b = min(g * 128 + 128, N)
            e0, e1 = rowptr[a], rowptr[b]
            cnt = e1 - e0
            idx[t, :cnt] = (s_srt[e0:e1] - BASE).astype(np.int16)
            dl[t, :cnt] = (d_srt[e0:e1] - g * 128).astype(np.float16)

        # rowptr pairs for degree (dummy nodes -> deg 1)
        node_ids = np.minimum(k * NPC + np.arange(NPC), N)
        ra = rowptr[node_ids].astype(np.int32)
        rb = rowptr[np.minimum(node_ids + 1, N)].astype(np.int32)

        xs = np.zeros((NPC, D), np.float32)
        nlo, nhi = k * NPC, min((k + 1) * NPC, N)
        xs[: nhi - nlo] = x[nlo:nhi]
        qs = np.zeros((QPC, D), np.float32)
        qlo, qhi = k * QPC, min((k + 1) * QPC, NQ)
        qs[: qhi - qlo] = q[qlo:qhi]

        ins.append({
            "x_sh": xs,
            "q_sh": qs,
            "w1": np.asarray(W1, np.float32),
            "w2": np.asarray(W2, np.float32),
            "wq": np.asarray(Wq, np.float32),
            "b1": np.asarray(b1, np.float32).reshape(1, D),
            "b2": np.asarray(b2, np.float32).reshape(1, D),
            "bq": np.asarray(bq, np.float32).reshape(1, D),
            "iota": iota16,
            "rpa": ra.reshape(NT, 128).T.copy(),
            "rpb": rb.reshape(NT, 128).T.copy(),
            "idx16": _wrap16(idx.reshape(-1), epad // 16),
            "dstloc": dl.reshape(-1).reshape(epad // 128, 128).T.copy(),
        })
    return cpt, ins


def _run(inputs, trace=False):
    cpt, ins = _prep(**inputs)
    import os
    key = (cpt, bool(os.environ.get("KERNEL_DEBUG")))
    if key not in _cache:
        _cache[key] = _build_nc(cpt)
    nc = _cache[key]
    res = run_bass_kernel_spmd(nc, ins, core_ids=list(range(N_CORES)),
                               trace=trace)
    h2 = np.concatenate(
        [res.results[k]["h2_out"][: min((k + 1) * NPC, N) - k * NPC]
         for k in range(N_CORES)])
    ques = np.concatenate(
        [res.results[k]["q_out"][: min((k + 1) * QPC, NQ) - k * QPC]
         for k in range(N_CORES)])
    return (ques, h2), res


def kernel(**inputs):
    out, _ = _run(inputs, trace=False)
    return out


def kernel_traced(**inputs):
    return _run(inputs, trace=True)


# revision 12
# speedup vs baseline: 2.0826x; 1.0180x over previous
"""GCN (2-layer GCNConv + question encoder) on 8 Trainium2 NeuronCores.

Strategy (self-contained, full inputs in / full outputs out):
  - Nodes partitioned contiguously across 8 cores (6272 = 49*128 per core,
    last core padded). Edges (with self-loops appended) are partitioned by
    destination node and sorted by destination on the host; each dst-tile of
    128 nodes gets its edge list padded to a uniform chunks-per-tile (CPT)
    so all cores run one SPMD program.
  - GCN identity used: D^-1/2 A D^-1/2 (h W) == scatter-add over edges of
    table[src] scaled by dinv[dst], where table = (dinv * h) @ W.
    The table is built shard-wise (dense matmul via a DMA-transposed fp16
    copy of dinv*h) and AllGathered to every core's HBM.
  - Per-edge gather uses the custom dma_gather instruction (int16 indices,
    base pointer placed mid-table so signed offsets cover all 50176 rows).
  - Scatter-add runs on the TensorEngine: per 128-edge chunk a one-hot
    [edge, dst_local] selection matrix (built by DVE iota-compare) is
    matmul-accumulated against the gathered messages in PSUM.
  - Epilogue per dst tile: out = psum * dinv[dst] + b (+ relu for layer 1),
    all f32. Messages/tables are fp16 (abs-max relative error ~2.6e-4).
  - The question encoder is data-parallel over rows, computed in f32 via
    PE-transpose + matmul, overlapped with the table AllGather.
"""
import sys

sys.path.insert(0, "/opt/trn_rl_repo")

import numpy as np
import ml_dtypes

import concourse.bacc as bacc
import concourse.mybir as mybir
import concourse.tile as tile
from concourse import bass
from concourse.bass_utils import run_bass_kernel_spmd
from concourse.library_config import mlp
from concourse.masks import make_identity

F32 = mybir.dt.float32
F16 = mybir.dt.float16
I16 = mybir.dt.int16
I32 = mybir.dt.int32

N_CORES = 8
N = 50000
D = 128
NT = 49                   # dst tiles per core
NPC = NT * 128            # nodes per core (padded)
TBL_ROWS = N_CORES * NPC  # 50176
BASE = 25000              # gather base row (signed int16 offsets)
GATHER_IDX_CAP = 5120     # smaller calls spread across 4 SWDGE queues
NQT = 20                  # q tiles per core
QPC = NQT * 128           # q rows per core (padded)
NQ = 20000

_cache = {}


def _build_nc(cpt: int):
    """Build and compile the SPMD kernel for a given chunks-per-tile."""
    epad = NT * cpt * 128  # padded edges per core

    nc = bacc.Bacc("TRN2", target_bir_lowering=False, debug=False,
                   num_devices=N_CORES, num_swdge_queues=4)
    dt = mybir.dt

    # ---- I/O -----------------------------------------------------------
    x_sh = nc.dram_tensor("x_sh", [NPC, D], F32, kind="ExternalInput")
    q_sh = nc.dram_tensor("q_sh", [QPC, D], F32, kind="ExternalInput")
    w1 = nc.dram_tensor("w1", [D, D], F32, kind="ExternalInput")
    w2 = nc.dram_tensor("w2", [D, D], F32, kind="ExternalInput")
    wq = nc.dram_tensor("wq", [D, D], F32, kind="ExternalInput")
    b1 = nc.dram_tensor("b1", [1, D], F32, kind="ExternalInput")
    b2 = nc.dram_tensor("b2", [1, D], F32, kind="ExternalInput")
    bq = nc.dram_tensor("bq", [1, D], F32, kind="ExternalInput")
    iota_in = nc.dram_tensor("iota", [128, 128], F16, kind="ExternalInput")
    rpa = nc.dram_tensor("rpa", [128, NT], I32, kind="ExternalInput")
    rpb = nc.dram_tensor("rpb", [128, NT], I32, kind="ExternalInput")
    idx16 = nc.dram_tensor("idx16", [128, epad // 16], I16, kind="ExternalInput")
    dstloc = nc.dram_tensor("dstloc", [128, epad // 128], F16, kind="ExternalInput")

    h2_out = nc.dram_tensor("h2_out", [NPC, D], F32, kind="ExternalOutput")
    q_out = nc.dram_tensor("q_out", [QPC, D], F32, kind="ExternalOutput")
    import os
    _dbg = bool(os.environ.get("KERNEL_DEBUG"))
    if _dbg:
        dbg_tbl = nc.dram_tensor("dbg_tbl", [TBL_ROWS, D], F16, kind="ExternalOutput")
        dbg_g1 = nc.dram_tensor("dbg_g1", [NPC, D], F16, kind="ExternalOutput")
        dbg_dinv = nc.dram_tensor("dbg_dinv", [128, NT], F32, kind="ExternalOutput")

    with tile.TileContext(nc) as tc:
        with (
            tc.tile_pool(name="const", bufs=1) as cpool,
            tc.tile_pool(name="work", bufs=3) as wpool,
            tc.tile_pool(name="gath", bufs=10) as gpool,
            tc.tile_pool(name="oh", bufs=5) as ohpool,
            tc.tile_pool(name="psA", bufs=4, space="PSUM") as psA,
            tc.tile_pool(name="psB", bufs=4, space="PSUM") as psB,
            tc.tile_pool(name="dram", bufs=1, space="DRAM") as dram,
        ):
            nc.gpsimd.load_library(mlp)

            # ---- constants ------------------------------------------------
            iota_t = cpool.tile([128, 128], F16)
            nc.sync.dma_start(iota_t[:], iota_in[:, :])
            ident = cpool.tile([128, 128], F32)
            make_identity(nc, ident[:])
            ones_row = cpool.tile([1, 128], F32)
            nc.vector.memset(ones_row[:], 1.0)

            def load_w16(w):
                wf = wpool.tile([128, 128], F32, name="wf32")
                nc.sync.dma_start(wf[:], w[:, :])
                wh = cpool.tile([128, 128], F16, name=f"wh_{w.name}")
                nc.vector.tensor_copy(wh[:], wf[:])
                return wh

            w1h = load_w16(w1)
            w2h = load_w16(w2)
            wq_t = cpool.tile([128, 128], F32)
            nc.sync.dma_start(wq_t[:], wq[:, :])

            def bias_bcast(b):
                brow = wpool.tile([1, 128], F32, name="brow")
                nc.sync.dma_start(brow[:], b[:, :])
                ps = psB.tile([128, 128], F32, space="PSUM", name="mps")
                nc.tensor.matmul(ps[:], lhsT=ones_row[:], rhs=brow[:],
                                 start=True, stop=True)
                bb = cpool.tile([128, 128], F32, name=f"bb_{b.name}")
                nc.vector.tensor_copy(bb[:], ps[:])
                return bb

            b1_bc = bias_bcast(b1)
            b2_bc = bias_bcast(b2)
            bq_bc = bias_bcast(bq)

            # ---- dinv = rsqrt(deg), deg = rowptr diff + 1 (self loop) -----
            ra = wpool.tile([128, NT], I32, name="ra")
            rb = wpool.tile([128, NT], I32, name="rb")
            nc.sync.dma_start(ra[:], rpa[:, :])
            nc.sync.dma_start(rb[:], rpb[:, :])
            degi = wpool.tile([128, NT], I32, name="degi")
            nc.vector.tensor_tensor(out=degi[:], in0=rb[:], in1=ra[:],
                                    op=mybir.AluOpType.subtract)
            # rowptr spans the self-loop-augmented edge list, so the diff
            # already includes the self loop — just convert int32 -> f32.
            deg = wpool.tile([128, NT], F32, name="degf")
            nc.vector.tensor_copy(deg[:], degi[:])
            dinv = cpool.tile([128, NT], F32)
            rec = wpool.tile([128, NT], F32, name="rec")
            nc.vector.reciprocal(rec[:], deg[:])
            nc.scalar.activation(dinv[:], rec[:],
                                 mybir.ActivationFunctionType.Sqrt)
            # two Newton iterations: y <- y*(1.5 - 0.5*d*y*y)
            for _ in range(2):
                t1 = wpool.tile([128, NT], F32, name="nwt1")
                nc.vector.tensor_tensor(out=t1[:], in0=dinv[:], in1=dinv[:],
                                        op=mybir.AluOpType.mult)
                nc.vector.tensor_tensor(out=t1[:], in0=t1[:], in1=deg[:],
                                        op=mybir.AluOpType.mult)
                nc.vector.tensor_scalar(out=t1[:], in0=t1[:], scalar1=-0.5,
                                        scalar2=1.5,
                                        op0=mybir.AluOpType.mult,
                                        op1=mybir.AluOpType.add)
                nc.vector.tensor_tensor(out=dinv[:], in0=dinv[:], in1=t1[:],
                                        op=mybir.AluOpType.mult)

            # ---- DRAM intermediates --------------------------------------
            g_hbm = [dram.tile([NPC, D], F16, name=f"g{i}") for i in range(2)]
            tbl_shard = [dram.tile([NPC, D], F16, name=f"ts{i}") for i in range(2)]
            tbl_full = [nc.dram_tensor(f"tf{i}", [TBL_ROWS, D], F16,
                                       addr_space="Shared").ap()
                        for i in range(2)]

            def build_table(layer, src_rows):
                """table = (dinv * src_rows) @ W, sharded + allgathered.

                src_rows: callable t -> AP of [128,128] f32 rows in SBUF
                (already scaled by dinv for layer 1; for layer 0 loads x)."""
                gh = g_hbm[layer]
                for t in range(NT):
                    g16 = wpool.tile([128, 128], F16, name="g16")
                    src_rows(t, g16)
                    nc.sync.dma_start(gh[t * 128:(t + 1) * 128, :], g16[:])
                gT = cpool.tile([128, NPC], F16, name=f"gT{layer}")
                nc.sync.dma_start(gT[:], gh[:, :], transpose=True)
                wh = w1h if layer == 0 else w2h
                for t in range(NT):
                    ps = psB.tile([128, 128], F32, space="PSUM", name="mps")
                    nc.tensor.matmul(ps[:], lhsT=gT[:, t * 128:(t + 1) * 128],
                                     rhs=wh[:], start=True, stop=True)
                    th = wpool.tile([128, 128], F16, name="th")
                    nc.vector.tensor_copy(th[:], ps[:])
                    nc.sync.dma_start(tbl_shard[layer][t * 128:(t + 1) * 128, :],
                                      th[:])
                nc.gpsimd.collective_compute(
                    "AllGather", mybir.AluOpType.bypass,
                    replica_groups=[list(range(N_CORES))],
                    ins=[tbl_shard[layer].opt()],
                    outs=[tbl_full[layer].opt()],
                )

            # ---- table for layer 1: (dinv * x) @ W1 ----------------------
            x_all = cpool.tile([128, NT * 128], F32)
            nc.sync.dma_start(
                x_all[:].rearrange("p (t d) -> p t d", d=128),
                x_sh.ap().rearrange("(t p) d -> p t d", p=128))

            def x_rows(t, out16):
                nc.vector.tensor_scalar(out=out16[:],
                                        in0=x_all[:, t * 128:(t + 1) * 128],
                                        scalar1=dinv[:, t:t + 1], scalar2=None,
                                        op0=mybir.AluOpType.mult)

            build_table(0, x_rows)

            # ---- question encoder (overlaps the AllGather) ---------------
            q_all = cpool.tile([128, NQT * 128], F32)
            nc.sync.dma_start(
                q_all[:].rearrange("p (t d) -> p t d", d=128),
                q_sh.ap().rearrange("(t p) d -> p t d", p=128))
            for t in range(NQT):
                qT_ps = psB.tile([128, 128], F32, space="PSUM", name="mps")
                nc.tensor.transpose(out=qT_ps[:],
                                    in_=q_all[:, t * 128:(t + 1) * 128],
                                    identity=ident[:])
                qT = wpool.tile([128, 128], F32, name="qT")
                nc.vector.tensor_copy(qT[:], qT_ps[:])
                qo_ps = psB.tile([128, 128], F32, space="PSUM", name="mps")
                nc.tensor.matmul(qo_ps[:], lhsT=qT[:], rhs=wq_t[:],
                                 start=True, stop=True)
                qo = wpool.tile([128, 128], F32, name="qo")
                nc.vector.tensor_tensor(out=qo[:], in0=qo_ps[:], in1=bq_bc[:],
                                        op=mybir.AluOpType.add)
                nc.sync.dma_start(q_out[t * 128:(t + 1) * 128, :], qo[:])

            # ---- load edge metadata (once) -------------------------------
            idx_sb = cpool.tile([128, epad // 16], I16)
            nc.sync.dma_start(idx_sb[:], idx16[:, :])
            dst_sb = cpool.tile([128, epad // 128], F16)
            nc.sync.dma_start(dst_sb[:], dstloc[:, :])

            # ---- GCN layers ----------------------------------------------
            PRE = 6  # gather calls issued ahead of their consumers

            def gcn_layer(layer):
                tbl = tbl_full[layer].tensor.ap()
                gt = max(1, GATHER_IDX_CAP // (cpt * 128))
                calls = [(g0, min(gt, NT - g0)) for g0 in range(0, NT, gt)]
                gath_tiles = {}

                def issue_gather(ci):
                    g0, ntile = calls[ci]
                    ncall = ntile * cpt * 128
                    gath = gpool.tile([128, gt * cpt, 128], F16, name="gath")
                    nc.gpsimd.dma_gather(
                        gath[:, :ntile * cpt, :],
                        tbl[BASE:, :],
                        idx_sb[:, (g0 * cpt * 128) // 16:
                                  ((g0 + ntile) * cpt * 128) // 16],
                        ncall, ncall, 128,
                        single_packet=False,
                        queue_num=ci % 4,
                    )
                    gath_tiles[ci] = gath

                def consume_call(ci):
                    g0, ntile = calls[ci]
                    gath = gath_tiles.pop(ci)
                    for ti in range(ntile):
                        t = g0 + ti
                        scatter_tile(layer, t, ti, gath)

                def scatter_tile(layer, t, ti, gath):
                        oh = ohpool.tile([128, cpt * 128], F16, name="oh")
                        nc.vector.tensor_tensor(
                            out=oh[:].rearrange("p (c j) -> p c j", j=128),
                            in0=dst_sb[:, t * cpt:(t + 1) * cpt]
                                .unsqueeze(-1).to_broadcast([128, cpt, 128]),
                            in1=iota_t[:].unsqueeze(1)
                                .to_broadcast([128, cpt, 128]),
                            op=mybir.AluOpType.is_equal,
                        )
                        ps = psA.tile([128, 128], F32, space="PSUM", name="agg")
                        for c in range(cpt):
                            nc.tensor.matmul(
                                ps[:],
                                lhsT=oh[:, c * 128:(c + 1) * 128],
                                rhs=gath[:, ti * cpt + c, :],
                                start=(c == 0), stop=(c == cpt - 1),
                            )
                        # epilogue
                        o = wpool.tile([128, 128], F32, name="epi_o")
                        nc.vector.tensor_scalar(
                            out=o[:], in0=ps[:], scalar1=dinv[:, t:t + 1],
                            scalar2=None, op0=mybir.AluOpType.mult)
                        bb = b1_bc if layer == 0 else b2_bc
                        nc.vector.tensor_tensor(out=o[:], in0=o[:], in1=bb[:],
                                                op=mybir.AluOpType.add)
                        if layer == 0:
                            h = wpool.tile([128, 128], F32, name="h1")
                            nc.scalar.activation(
                                h[:], o[:], mybir.ActivationFunctionType.Relu)
                            g16 = wpool.tile([128, 128], F16, name="g16b")
                            nc.vector.tensor_scalar(
                                out=g16[:], in0=h[:],
                                scalar1=dinv[:, t:t + 1], scalar2=None,
                                op0=mybir.AluOpType.mult)
                            nc.sync.dma_start(
                                g_hbm[1][t * 128:(t + 1) * 128, :], g16[:])
                        else:
                            nc.sync.dma_start(
                                h2_out[t * 128:(t + 1) * 128, :], o[:])

                for ci in range(len(calls) + PRE):
                    if ci < len(calls):
                        issue_gather(ci)
                    if ci >= PRE:
                        consume_call(ci - PRE)

            gcn_layer(0)

            if _dbg:
                dbt = wpool.tile([128, NT], F32, name="dbt")
                nc.vector.tensor_copy(dbt[:], dinv[:])
                nc.sync.dma_start(dbg_dinv[:, :], dbt[:])
                for t in range(0, TBL_ROWS // 128):
                    tt = wpool.tile([128, 128], F16, name="dtt")
                    nc.sync.dma_start(tt[:], tbl_full[0].tensor.ap()[t * 128:(t + 1) * 128, :])
                    nc.sync.dma_start(dbg_tbl[t * 128:(t + 1) * 128, :], tt[:])
                for t in range(NT):
                    tg = wpool.tile([128, 128], F16, name="dtg")
                    nc.sync.dma_start(tg[:], g_hbm[1][t * 128:(t + 1) * 128, :])
                    nc.sync.dma_start(dbg_g1[t * 128:(t + 1) * 128, :], tg[:])

            # table for layer 2 from g_hbm[1] (already dinv*relu(h1), f16)
            gT2 = cpool.tile([128, NPC], F16)
            nc.sync.dma_start(gT2[:], g_hbm[1][:, :], transpose=True)
            for t in range(NT):
                ps = psB.tile([128, 128], F32, space="PSUM", name="mps")
                nc.tensor.matmul(ps[:], lhsT=gT2[:, t * 128:(t + 1) * 128],
                                 rhs=w2h[:], start=True, stop=True)
                th = wpool.tile([128, 128], F16, name="th2")
                nc.vector.tensor_copy(th[:], ps[:])
                nc.sync.dma_start(tbl_shard[1][t * 128:(t + 1) * 128, :], th[:])
            nc.gpsimd.collective_compute(
                "AllGather", mybir.AluOpType.bypass,
                replica_groups=[list(range(N_CORES))],
                ins=[tbl_shard[1].opt()],
                outs=[tbl_full[1].opt()],
            )

            gcn_layer(1)

    nc.compile()
    return nc


def _wrap16(a, cols):
    """[n] int16 -> [128, n/16]: element i at partition i%16 (replicated x8)."""
    w = a.reshape(cols, 16).T
    return np.tile(w, (8, 1)).copy()


def _prep(x, edge_index, q_emb, W1, b1, W2, b2, Wq, bq):
    src = np.asarray(edge_index[0], dtype=np.int64)
    dst = np.asarray(edge_index[1], dtype=np.int64)
    # self loops as real edges
    loop = np.arange(N, dtype=np.int64)
    s_all = np.concatenate([src, loop])
    d_all = np.concatenate([dst, loop])
    order = np.argsort(d_all, kind="stable")
    s_srt = s_all[order]
    d_srt = d_all[order]
    rowptr = np.searchsorted(d_srt, np.arange(N + 1))

    # per-global-tile edge counts -> uniform chunks per tile (always >=1 pad)
    gt_starts = rowptr[np.minimum(np.arange(0, TBL_ROWS + 1, 128), N)]
    counts = gt_starts[1:] - gt_starts[:-1]  # length 392 incl dummy tiles
    cpt = int(np.max(counts) // 128 + 1)
    epad = NT * cpt * 128

    ins = []
    x = np.asarray(x, dtype=np.float32)
    q = np.asarray(q_emb, dtype=np.float32)
    iota = np.broadcast_to(np.arange(128, dtype=np.float32), (128, 128))
    iota16 = iota.astype(np.float16)

    for k in range(N_CORES):
        idx = np.zeros((NT, cpt * 128), dtype=np.int16)
        dl = np.full((NT, cpt * 128), -1.0, dtype=np.float16)
        for t in range(NT):
            g = k * NT + t
            a = min(g * 128, N)
            b = min(g * 128 + 128, N)
            e0, e1 = rowptr[a], rowptr[b]
            cnt = e1 - e0
            idx[t, :cnt] = (s_srt[e0:e1] - BASE).astype(np.int16)
            dl[t, :cnt] = (d_srt[e0:e1] - g * 128).astype(np.float16)

        # rowptr pairs for degree (dummy nodes -> deg 1)
        node_ids = np.minimum(k * NPC + np.arange(NPC), N)
        ra = rowptr[node_ids].astype(np.int32)
        rb = rowptr[np.minimum(node_ids + 1, N)].astype(np.int32)

        xs = np.zeros((NPC, D), np.float32)
        nlo, nhi = k * NPC, min((k + 1) * NPC, N)
        xs[: nhi - nlo] = x[nlo:nhi]
        qs = np.zeros((QPC, D), np.float32)
        qlo, qhi = k * QPC, min((k + 1) * QPC, NQ)
        qs[: qhi - qlo] = q[qlo:qhi]

        ins.append({
            "x_sh": xs,
            "q_sh": qs,
            "w1": np.asarray(W1, np.float32),
            "w2": np.asarray(W2, np.float32),
            "wq": np.asarray(Wq, np.float32),
            "b1": np.asarray(b1, np.float32).reshape(1, D),
            "b2": np.asarray(b2, np.float32).reshape(1, D),
            "bq": np.asarray(bq, np.float32).reshape(1, D),
            "iota": iota16,
            "rpa": ra.reshape(NT, 128).T.copy(),
            "rpb": rb.reshape(NT, 128).T.copy(),
            "idx16": _wrap16(idx.reshape(-1), epad // 16),
            "dstloc": dl.reshape(-1).reshape(epad // 128, 128).T.copy(),
        })
    return cpt, ins


def _run(inputs, trace=False):
    cpt, ins = _prep(**inputs)
    import os
    key = (cpt, bool(os.environ.get("KERNEL_DEBUG")))
    if key not in _cache:
        _cache[key] = _build_nc(cpt)
    nc = _cache[key]
    res = run_bass_kernel_spmd(nc, ins, core_ids=list(range(N_CORES)),
                               trace=trace)
    h2 = np.concatenate(
        [res.results[k]["h2_out"][: min((k + 1) * NPC, N) - k * NPC]
         for k in range(N_CORES)])
    ques = np.concatenate(
        [res.results[k]["q_out"][: min((k + 1) * QPC, NQ) - k * QPC]
         for k in range(N_CORES)])
    return (ques, h2), res


def kernel(**inputs):
    out, _ = _run(inputs, trace=False)
    return out


def kernel_traced(**inputs):
    return _run(inputs, trace=True)


# revision 13
# speedup vs baseline: 2.5276x; 1.2137x over previous
"""GCN (2-layer GCNConv + question encoder) on 8 Trainium2 NeuronCores.

Strategy (self-contained, full inputs in / full outputs out):
  - Nodes partitioned contiguously across 8 cores (6272 = 49*128 per core,
    last core padded). Edges (with self-loops appended) are partitioned by
    destination node and sorted by destination on the host; each dst-tile of
    128 nodes gets its edge list padded to a uniform chunks-per-tile (CPT)
    so all cores run one SPMD program.
  - GCN identity used: D^-1/2 A D^-1/2 (h W) == scatter-add over edges of
    table[src] scaled by dinv[dst], where table = (dinv * h) @ W.
    The table is built shard-wise (dense matmul via a DMA-transposed fp16
    copy of dinv*h) and AllGathered to every core's HBM.
  - Per-edge gather uses the custom dma_gather instruction (int16 indices,
    base pointer placed mid-table so signed offsets cover all 50176 rows).
  - Scatter-add runs on the TensorEngine: per 128-edge chunk a one-hot
    [edge, dst_local] selection matrix (built by DVE iota-compare) is
    matmul-accumulated against the gathered messages in PSUM.
  - Epilogue per dst tile: out = psum * dinv[dst] + b (+ relu for layer 1),
    all f32. Messages/tables are fp16 (abs-max relative error ~2.6e-4).
  - The question encoder is data-parallel over rows, computed in f32 via
    PE-transpose + matmul, overlapped with the table AllGather.
"""
import sys

sys.path.insert(0, "/opt/trn_rl_repo")

import numpy as np
import ml_dtypes

import concourse.bacc as bacc
import concourse.mybir as mybir
import concourse.tile as tile
from concourse import bass
from concourse.bass_utils import run_bass_kernel_spmd
from concourse.library_config import mlp
from concourse.masks import make_identity

F32 = mybir.dt.float32
F16 = mybir.dt.float16
I16 = mybir.dt.int16
I32 = mybir.dt.int32

N_CORES = 8
N = 50000
D = 128
NT = 49                   # dst tiles per core
NPC = NT * 128            # nodes per core (padded)
TBL_ROWS = N_CORES * NPC  # 50176
BASE = 25000              # gather base row (signed int16 offsets)
GATHER_IDX_CAP = 2560     # one dst-tile per gather call
NQT = 20                  # q tiles per core
QPC = NQT * 128           # q rows per core (padded)
NQ = 20000

_cache = {}


def _build_nc(cpt: int):
    """Build and compile the SPMD kernel for a given chunks-per-tile."""
    epad = NT * cpt * 128  # padded edges per core

    nc = bacc.Bacc("TRN2", target_bir_lowering=False, debug=False,
                   num_devices=N_CORES, num_swdge_queues=4)
    dt = mybir.dt

    # ---- I/O -----------------------------------------------------------
    x_sh = nc.dram_tensor("x_sh", [NPC, D], F32, kind="ExternalInput")
    q_sh = nc.dram_tensor("q_sh", [QPC, D], F32, kind="ExternalInput")
    w1 = nc.dram_tensor("w1", [D, D], F32, kind="ExternalInput")
    w2 = nc.dram_tensor("w2", [D, D], F32, kind="ExternalInput")
    wq = nc.dram_tensor("wq", [D, D], F32, kind="ExternalInput")
    b1 = nc.dram_tensor("b1", [1, D], F32, kind="ExternalInput")
    b2 = nc.dram_tensor("b2", [1, D], F32, kind="ExternalInput")
    bq = nc.dram_tensor("bq", [1, D], F32, kind="ExternalInput")
    iota_in = nc.dram_tensor("iota", [128, 128], F16, kind="ExternalInput")
    rpa = nc.dram_tensor("rpa", [128, NT], I32, kind="ExternalInput")
    rpb = nc.dram_tensor("rpb", [128, NT], I32, kind="ExternalInput")
    idx16 = nc.dram_tensor("idx16", [128, epad // 16], I16, kind="ExternalInput")
    dstloc = nc.dram_tensor("dstloc", [128, epad // 128], F16, kind="ExternalInput")

    h2_out = nc.dram_tensor("h2_out", [NPC, D], F32, kind="ExternalOutput")
    q_out = nc.dram_tensor("q_out", [QPC, D], F32, kind="ExternalOutput")
    import os
    _dbg = bool(os.environ.get("KERNEL_DEBUG"))
    if _dbg:
        dbg_tbl = nc.dram_tensor("dbg_tbl", [TBL_ROWS, D], F16, kind="ExternalOutput")
        dbg_g1 = nc.dram_tensor("dbg_g1", [NPC, D], F16, kind="ExternalOutput")
        dbg_dinv = nc.dram_tensor("dbg_dinv", [128, NT], F32, kind="ExternalOutput")

    with tile.TileContext(nc) as tc:
        with (
            tc.tile_pool(name="const", bufs=1) as cpool,
            tc.tile_pool(name="work", bufs=3) as wpool,
            tc.tile_pool(name="gath", bufs=20) as gpool,
            tc.tile_pool(name="oh", bufs=5) as ohpool,
            tc.tile_pool(name="psA", bufs=5, space="PSUM") as psA,
            tc.tile_pool(name="psB", bufs=3, space="PSUM") as psB,
            tc.tile_pool(name="dram", bufs=1, space="DRAM") as dram,
        ):
            nc.gpsimd.load_library(mlp)

            # ---- constants ------------------------------------------------
            iota_t = cpool.tile([128, 128], F16)
            nc.sync.dma_start(iota_t[:], iota_in[:, :])
            ident = cpool.tile([128, 128], F32)
            make_identity(nc, ident[:])
            ones_row = cpool.tile([1, 128], F32)
            nc.vector.memset(ones_row[:], 1.0)

            def load_w16(w):
                wf = wpool.tile([128, 128], F32, name="wf32")
                nc.sync.dma_start(wf[:], w[:, :])
                wh = cpool.tile([128, 128], F16, name=f"wh_{w.name}")
                nc.vector.tensor_copy(wh[:], wf[:])
                return wh

            w1h = load_w16(w1)
            w2h = load_w16(w2)
            wq_t = cpool.tile([128, 128], F32)
            nc.sync.dma_start(wq_t[:], wq[:, :])

            def bias_bcast(b):
                brow = wpool.tile([1, 128], F32, name="brow")
                nc.sync.dma_start(brow[:], b[:, :])
                ps = psB.tile([128, 128], F32, space="PSUM", name="mps")
                nc.tensor.matmul(ps[:], lhsT=ones_row[:], rhs=brow[:],
                                 start=True, stop=True)
                bb = cpool.tile([128, 128], F32, name=f"bb_{b.name}")
                nc.vector.tensor_copy(bb[:], ps[:])
                return bb

            b1_bc = bias_bcast(b1)
            b2_bc = bias_bcast(b2)
            bq_bc = bias_bcast(bq)

            # ---- dinv = rsqrt(deg), deg = rowptr diff + 1 (self loop) -----
            ra = wpool.tile([128, NT], I32, name="ra")
            rb = wpool.tile([128, NT], I32, name="rb")
            nc.sync.dma_start(ra[:], rpa[:, :])
            nc.sync.dma_start(rb[:], rpb[:, :])
            degi = wpool.tile([128, NT], I32, name="degi")
            nc.vector.tensor_tensor(out=degi[:], in0=rb[:], in1=ra[:],
                                    op=mybir.AluOpType.subtract)
            # rowptr spans the self-loop-augmented edge list, so the diff
            # already includes the self loop — just convert int32 -> f32.
            deg = wpool.tile([128, NT], F32, name="degf")
            nc.vector.tensor_copy(deg[:], degi[:])
            dinv = cpool.tile([128, NT], F32)
            rec = wpool.tile([128, NT], F32, name="rec")
            nc.vector.reciprocal(rec[:], deg[:])
            nc.scalar.activation(dinv[:], rec[:],
                                 mybir.ActivationFunctionType.Sqrt)
            # two Newton iterations: y <- y*(1.5 - 0.5*d*y*y)
            for _ in range(2):
                t1 = wpool.tile([128, NT], F32, name="nwt1")
                nc.vector.tensor_tensor(out=t1[:], in0=dinv[:], in1=dinv[:],
                                        op=mybir.AluOpType.mult)
                nc.vector.tensor_tensor(out=t1[:], in0=t1[:], in1=deg[:],
                                        op=mybir.AluOpType.mult)
                nc.vector.tensor_scalar(out=t1[:], in0=t1[:], scalar1=-0.5,
                                        scalar2=1.5,
                                        op0=mybir.AluOpType.mult,
                                        op1=mybir.AluOpType.add)
                nc.vector.tensor_tensor(out=dinv[:], in0=dinv[:], in1=t1[:],
                                        op=mybir.AluOpType.mult)

            # ---- DRAM intermediates --------------------------------------
            g_hbm = [dram.tile([NPC, D], F16, name=f"g{i}") for i in range(2)]
            tbl_shard = [dram.tile([NPC, D], F16, name=f"ts{i}") for i in range(2)]
            tbl_full = [nc.dram_tensor(f"tf{i}", [TBL_ROWS, D], F16,
                                       addr_space="Shared").ap()
                        for i in range(2)]

            def build_table(layer, src_rows):
                """table = (dinv * src_rows) @ W, sharded + allgathered.

                src_rows: callable t -> AP of [128,128] f32 rows in SBUF
                (already scaled by dinv for layer 1; for layer 0 loads x)."""
                gh = g_hbm[layer]
                for t in range(NT):
                    g16 = wpool.tile([128, 128], F16, name="g16")
                    src_rows(t, g16)
                    nc.sync.dma_start(gh[t * 128:(t + 1) * 128, :], g16[:])
                gT = cpool.tile([128, NPC], F16, name="gT", bufs=1)
                nc.sync.dma_start(gT[:], gh[:, :], transpose=True)
                wh = w1h if layer == 0 else w2h
                for t in range(NT):
                    ps = psB.tile([128, 128], F32, space="PSUM", name="mps")
                    nc.tensor.matmul(ps[:], lhsT=gT[:, t * 128:(t + 1) * 128],
                                     rhs=wh[:], start=True, stop=True)
                    th = wpool.tile([128, 128], F16, name="th")
                    nc.vector.tensor_copy(th[:], ps[:])
                    nc.sync.dma_start(tbl_shard[layer][t * 128:(t + 1) * 128, :],
                                      th[:])
                nc.gpsimd.collective_compute(
                    "AllGather", mybir.AluOpType.bypass,
                    replica_groups=[list(range(N_CORES))],
                    ins=[tbl_shard[layer].opt()],
                    outs=[tbl_full[layer].opt()],
                )

            # ---- table for layer 1: (dinv * x) @ W1 ----------------------
            def x_rows(t, out16):
                xt = wpool.tile([128, 128], F32, name="xt")
                nc.sync.dma_start(xt[:], x_sh[t * 128:(t + 1) * 128, :])
                nc.scalar.activation(out16[:], xt[:],
                                     mybir.ActivationFunctionType.Copy,
                                     scale=dinv[:, t:t + 1])

            build_table(0, x_rows)

            # ---- question encoder (overlaps the AllGather) ---------------
            for t in range(NQT):
                qt = wpool.tile([128, 128], F32, name="qt")
                nc.sync.dma_start(qt[:], q_sh[t * 128:(t + 1) * 128, :])
                qT_ps = psB.tile([128, 128], F32, space="PSUM", name="mps")
                nc.tensor.transpose(out=qT_ps[:], in_=qt[:], identity=ident[:])
                qT = wpool.tile([128, 128], F32, name="qT")
                nc.vector.tensor_copy(qT[:], qT_ps[:])
                qo_ps = psB.tile([128, 128], F32, space="PSUM", name="mps")
                nc.tensor.matmul(qo_ps[:], lhsT=qT[:], rhs=wq_t[:],
                                 start=True, stop=True)
                qo = wpool.tile([128, 128], F32, name="qo")
                nc.vector.tensor_tensor(out=qo[:], in0=qo_ps[:], in1=bq_bc[:],
                                        op=mybir.AluOpType.add)
                nc.sync.dma_start(q_out[t * 128:(t + 1) * 128, :], qo[:])

            # ---- load edge metadata (once) -------------------------------
            idx_sb = cpool.tile([128, epad // 16], I16)
            nc.sync.dma_start(idx_sb[:], idx16[:, :])
            dst_sb = cpool.tile([128, epad // 128], F16)
            nc.sync.dma_start(dst_sb[:], dstloc[:, :])

            # ---- GCN layers ----------------------------------------------
            PRE = 10  # gather calls issued ahead of their consumers

            def gcn_layer(layer):
                tbl = tbl_full[layer].tensor.ap()
                gt = max(1, GATHER_IDX_CAP // (cpt * 128))
                calls = [(g0, min(gt, NT - g0)) for g0 in range(0, NT, gt)]
                gath_tiles = {}

                def issue_gather(ci):
                    g0, ntile = calls[ci]
                    ncall = ntile * cpt * 128
                    gath = gpool.tile([128, gt * cpt, 128], F16, name="gath")
                    nc.gpsimd.dma_gather(
                        gath[:, :ntile * cpt, :],
                        tbl[BASE:, :],
                        idx_sb[:, (g0 * cpt * 128) // 16:
                                  ((g0 + ntile) * cpt * 128) // 16],
                        ncall, ncall, 128,
                        single_packet=False,
                        queue_num=ci % 4,
                    )
                    gath_tiles[ci] = gath

                def consume_call(ci):
                    g0, ntile = calls[ci]
                    gath = gath_tiles.pop(ci)
                    for ti in range(ntile):
                        t = g0 + ti
                        scatter_tile(layer, t, ti, gath)

                def scatter_tile(layer, t, ti, gath):
                        oh = ohpool.tile([128, cpt * 128], F16, name="oh")
                        nc.vector.tensor_tensor(
                            out=oh[:].rearrange("p (c j) -> p c j", j=128),
                            in0=dst_sb[:, t * cpt:(t + 1) * cpt]
                                .unsqueeze(-1).to_broadcast([128, cpt, 128]),
                            in1=iota_t[:].unsqueeze(1)
                                .to_broadcast([128, cpt, 128]),
                            op=mybir.AluOpType.is_equal,
                        )
                        ps = psA.tile([128, 128], F32, space="PSUM", name="agg")
                        for c in range(cpt):
                            nc.tensor.matmul(
                                ps[:],
                                lhsT=oh[:, c * 128:(c + 1) * 128],
                                rhs=gath[:, ti * cpt + c, :],
                                start=(c == 0), stop=(c == cpt - 1),
                            )
                        # epilogue: out = psum*dinv + b (+relu, *dinv for L0)
                        o = wpool.tile([128, 128], F32, name="epi_o")
                        nc.scalar.activation(o[:], ps[:],
                                             mybir.ActivationFunctionType.Copy,
                                             scale=dinv[:, t:t + 1])
                        bb = b1_bc if layer == 0 else b2_bc
                        nc.vector.tensor_tensor(out=o[:], in0=o[:], in1=bb[:],
                                                op=mybir.AluOpType.add)
                        if layer == 0:
                            # g1 = dinv*relu(o) == relu(dinv*o) since dinv>0
                            g16 = wpool.tile([128, 128], F16, name="g16b")
                            nc.scalar.activation(
                                g16[:], o[:],
                                mybir.ActivationFunctionType.Relu,
                                scale=dinv[:, t:t + 1])
                            nc.sync.dma_start(
                                g_hbm[1][t * 128:(t + 1) * 128, :], g16[:])
                        else:
                            nc.sync.dma_start(
                                h2_out[t * 128:(t + 1) * 128, :], o[:])

                for ci in range(len(calls) + PRE):
                    if ci < len(calls):
                        issue_gather(ci)
                    if ci >= PRE:
                        consume_call(ci - PRE)

            gcn_layer(0)

            if _dbg:
                dbt = wpool.tile([128, NT], F32, name="dbt")
                nc.vector.tensor_copy(dbt[:], dinv[:])
                nc.sync.dma_start(dbg_dinv[:, :], dbt[:])
                for t in range(0, TBL_ROWS // 128):
                    tt = wpool.tile([128, 128], F16, name="dtt")
                    nc.sync.dma_start(tt[:], tbl_full[0].tensor.ap()[t * 128:(t + 1) * 128, :])
                    nc.sync.dma_start(dbg_tbl[t * 128:(t + 1) * 128, :], tt[:])
                for t in range(NT):
                    tg = wpool.tile([128, 128], F16, name="dtg")
                    nc.sync.dma_start(tg[:], g_hbm[1][t * 128:(t + 1) * 128, :])
                    nc.sync.dma_start(dbg_g1[t * 128:(t + 1) * 128, :], tg[:])

            # table for layer 2 from g_hbm[1] (already dinv*relu(h1), f16)
            gT2 = cpool.tile([128, NPC], F16, name="gT", bufs=1)
            nc.sync.dma_start(gT2[:], g_hbm[1][:, :], transpose=True)
            for t in range(NT):
                ps = psB.tile([128, 128], F32, space="PSUM", name="mps")
                nc.tensor.matmul(ps[:], lhsT=gT2[:, t * 128:(t + 1) * 128],
                                 rhs=w2h[:], start=True, stop=True)
                th = wpool.tile([128, 128], F16, name="th2")
                nc.vector.tensor_copy(th[:], ps[:])
                nc.sync.dma_start(tbl_shard[1][t * 128:(t + 1) * 128, :], th[:])
            nc.gpsimd.collective_compute(
                "AllGather", mybir.AluOpType.bypass,
                replica_groups=[list(range(N_CORES))],
                ins=[tbl_shard[1].opt()],
                outs=[tbl_full[1].opt()],
            )

            gcn_layer(1)

    nc.compile()
    return nc


def _wrap16(a, cols):
    """[n] int16 -> [128, n/16]: element i at partition i%16 (replicated x8)."""
    w = a.reshape(cols, 16).T
    return np.tile(w, (8, 1)).copy()


def _prep(x, edge_index, q_emb, W1, b1, W2, b2, Wq, bq):
    src = np.asarray(edge_index[0], dtype=np.int64)
    dst = np.asarray(edge_index[1], dtype=np.int64)
    # self loops as real edges
    loop = np.arange(N, dtype=np.int64)
    s_all = np.concatenate([src, loop])
    d_all = np.concatenate([dst, loop])
    order = np.argsort(d_all, kind="stable")
    s_srt = s_all[order]
    d_srt = d_all[order]
    rowptr = np.searchsorted(d_srt, np.arange(N + 1))

    # per-global-tile edge counts -> uniform chunks per tile (always >=1 pad)
    gt_starts = rowptr[np.minimum(np.arange(0, TBL_ROWS + 1, 128), N)]
    counts = gt_starts[1:] - gt_starts[:-1]  # length 392 incl dummy tiles
    cpt = int(np.max(counts) // 128 + 1)
    epad = NT * cpt * 128

    ins = []
    x = np.asarray(x, dtype=np.float32)
    q = np.asarray(q_emb, dtype=np.float32)
    iota = np.broadcast_to(np.arange(128, dtype=np.float32), (128, 128))
    iota16 = iota.astype(np.float16)

    for k in range(N_CORES):
        idx = np.zeros((NT, cpt * 128), dtype=np.int16)
        dl = np.full((NT, cpt * 128), -1.0, dtype=np.float16)
        for t in range(NT):
            g = k * NT + t
            a = min(g * 128, N)
            b = min(g * 128 + 128, N)
            e0, e1 = rowptr[a], rowptr[b]
            cnt = e1 - e0
            idx[t, :cnt] = (s_srt[e0:e1] - BASE).astype(np.int16)
            dl[t, :cnt] = (d_srt[e0:e1] - g * 128).astype(np.float16)

        # rowptr pairs for degree (dummy nodes -> deg 1)
        node_ids = np.minimum(k * NPC + np.arange(NPC), N)
        ra = rowptr[node_ids].astype(np.int32)
        rb = rowptr[np.minimum(node_ids + 1, N)].astype(np.int32)

        xs = np.zeros((NPC, D), np.float32)
        nlo, nhi = k * NPC, min((k + 1) * NPC, N)
        xs[: nhi - nlo] = x[nlo:nhi]
        qs = np.zeros((QPC, D), np.float32)
        qlo, qhi = k * QPC, min((k + 1) * QPC, NQ)
        qs[: qhi - qlo] = q[qlo:qhi]

        ins.append({
            "x_sh": xs,
            "q_sh": qs,
            "w1": np.asarray(W1, np.float32),
            "w2": np.asarray(W2, np.float32),
            "wq": np.asarray(Wq, np.float32),
            "b1": np.asarray(b1, np.float32).reshape(1, D),
            "b2": np.asarray(b2, np.float32).reshape(1, D),
            "bq": np.asarray(bq, np.float32).reshape(1, D),
            "iota": iota16,
            "rpa": ra.reshape(NT, 128).T.copy(),
            "rpb": rb.reshape(NT, 128).T.copy(),
            "idx16": _wrap16(idx.reshape(-1), epad // 16),
            "dstloc": dl.reshape(-1).reshape(epad // 128, 128).T.copy(),
        })
    return cpt, ins


def _run(inputs, trace=False):
    cpt, ins = _prep(**inputs)
    import os
    key = (cpt, bool(os.environ.get("KERNEL_DEBUG")))
    if key not in _cache:
        _cache[key] = _build_nc(cpt)
    nc = _cache[key]
    res = run_bass_kernel_spmd(nc, ins, core_ids=list(range(N_CORES)),
                               trace=trace)
    h2 = np.concatenate(
        [res.results[k]["h2_out"][: min((k + 1) * NPC, N) - k * NPC]
         for k in range(N_CORES)])
    ques = np.concatenate(
        [res.results[k]["q_out"][: min((k + 1) * QPC, NQ) - k * QPC]
         for k in range(N_CORES)])
    return (ques, h2), res


def kernel(**inputs):
    out, _ = _run(inputs, trace=False)
    return out


def kernel_traced(**inputs):
    return _run(inputs, trace=True)
